# revision 1
# baseline (speedup 1.0000x reference)
"""Trainium2 Bass kernel for nn_DelayedSelfAttention (B=4, T=1024, C=1024, H=16).

Sharding: 8 cores = 4 batches x 2 sequence-halves.  Core c handles batch
c//2 and query rows [r*T, (r+1)*T) of the concatenated [2T] sequence
(r = c%2).  Each core computes K/V for the full 2T sequence (duplicated
kv-projection instead of any collective), attention for its T query rows
over all 16 heads, and the output projection for its rows.  Role
asymmetry (mask values, q/proj LoRA) is pushed into per-core input data
so a single SPMD program serves all cores.

Attention runs in the S^T orientation (keys on partitions, queries on
the free axis): no transposes anywhere.  exp on ScalarE, multiplicative
{0,1} masks on boundary tiles only, AV via V augmented with a ones
column so the softmax denominator accumulates as row 64 of the [65, q]
matmul output and never leaves PSUM.  QKV-projection inputs and the
K/V/Q/P attention operands are bf16 (halves DMA; rel err ~4e-3); the
y/W_proj output projection runs in float32r (full-rate fp32 path).
Softmax skips max-subtraction (scores are O(1) by construction).
"""

import contextlib
import sys

for _p in ("/opt/trn_rl_repo", "/root/.axon_site/_ro/trn_rl_repo"):
    if _p not in sys.path:
        sys.path.insert(0, _p)

import ml_dtypes
import numpy as np

import concourse.bass as bass
import concourse.mybir as mybir
import concourse.tile as tile_mod
from concourse.bass_utils import run_bass_kernel_spmd
from concourse.tile import TileContext
from concourse.vector_clock import ScopedClock

# ---------------------------------------------------------------------------
# Workaround: this walrus build supports a single semaphore wait per
# instruction.  Split multi-wait instructions into same-engine NoOps each
# carrying one wait (identical sequencer semantics).
# ---------------------------------------------------------------------------
_ws_counter = [0]


def _fresh_name():
    _ws_counter[0] += 1
    return f"I-waitsplit-{_ws_counter[0]}"


def _split_inst_waits(inst):
    si = inst.sync_info
    if si is None:
        return []
    waits = list(si.on_wait or [])
    if len(waits) <= 1:
        return []
    nops = []
    for w in waits[:-1]:
        nop = mybir.InstNoOp(name=_fresh_name())
        nop.engine = inst.engine
        nop.sync_info = mybir.SyncInfo(on_wait=[w], on_update=[])
        nops.append(nop)
    inst.sync_info = mybir.SyncInfo(
        on_wait=[waits[-1]], on_update=list(si.on_update or [])
    )
    return nops


_orig_lower = tile_mod.TileContext._lower_ordered_insts


def _patched_lower(self, ordered):
    for bb_name in list(ordered.keys()):
        new = []
        for inst in ordered[bb_name]:
            new.extend(_split_inst_waits(inst))
            new.append(inst)
        ordered[bb_name] = new
    return _orig_lower(self, ordered)


def _patched_drain_and_barrier(self, tick_clock, wait_clock):
    nc = self.nc
    drain_inst = nc.sync.drain()
    wait_clock.add_sem_waits(
        drain_inst.ins, ScopedClock({None: tick_clock.global_clock})
    )
    nops = _split_inst_waits(drain_inst.ins)
    if nops:
        first_wait = drain_inst.ins.sync_info
        drain_inst.ins.sync_info = mybir.SyncInfo(on_wait=[], on_update=[])
        for nop in nops:
            n2 = nc.sync.nop(nofuse=True)
            n2.ins.sync_info = nop.sync_info
        d2 = nc.sync.drain()
        d2.ins.sync_info = first_wait

    nc.all_engine_barrier()
    assert self.sems is not None
    popped = nc._tile_sem_poison_stack.pop()
    assert popped is self._sem_poison
    nc.clear_and_free_semaphores(list(self.sems.allocated().values()))
    nc.all_engine_barrier()


def _apply_tile_patch():
    if tile_mod.TileContext._lower_ordered_insts is not _patched_lower:
        tile_mod.TileContext._lower_ordered_insts = _patched_lower
        tile_mod.TileContext._drain_and_barrier = _patched_drain_and_barrier


# ---------------------------------------------------------------------------
# Problem constants (hardcoded per the task contract).
# ---------------------------------------------------------------------------
B, T, C, H = 4, 1024, 1024, 16
D = C // H  # 64
SEQ = 2 * T
LOOKAHEAD, OVERLAP = 64, 64
RANK, ALPHA = 8, 16.0
LSCALE = ALPHA / RANK  # 2.0
QSCALE = 1.0 / np.sqrt(D)  # 1/8
NCH = C // 128  # 8 c-chunks
NQT = T // 128  # 8 q-subtiles per core
F32 = mybir.dt.float32
F32R = mybir.dt.float32r
BF16 = mybir.dt.bfloat16


def _r(ap):
    return ap


# Trace-time tiling structure, shared by host (mask packing) and device.
def _ktiles_for_qblock(qb):
    """k-tiles (region, j) touched by q-subtiles [4qb, 4qb+4)."""
    qts = range(4 * qb, 4 * qb + 4)
    e1 = sorted({j for qt in qts for j in (qt - 1, qt, qt + 1) if 0 <= j < NQT})
    e2 = sorted({j for qt in qts for j in range(qt + 1)})
    return [("e1", j) for j in e1] + [("e2", j) for j in e2]


def _active_qts(region, j, qb):
    if region == "e1":
        qts = [qt for qt in range(4 * qb, 4 * qb + 4) if j in (qt - 1, qt, qt + 1)]
    else:
        qts = [qt for qt in range(4 * qb, 4 * qb + 4) if j <= qt]
    assert qts == list(range(qts[0], qts[-1] + 1))
    return qts


def _mask_tiles():
    out = []
    for qt in range(NQT):
        for j in (qt - 1, qt, qt + 1):
            if 0 <= j < NQT:
                out.append(("e1", j, qt))
        for j in (qt - 1, qt):
            if j >= 0:
                out.append(("e2", j, qt))
    return out


MASK_TILES = _mask_tiles()  # 37 tiles
MASK_IDX = {k: i for i, k in enumerate(MASK_TILES)}
NMASK = len(MASK_TILES)


def _accum(nc, out_ps, pairs):
    """Accumulating matmul group: list of (lhsT, rhs) into one psum tile."""
    n = len(pairs)
    for i, (lh, rh) in enumerate(pairs):
        nc.tensor.matmul(out_ps, _r(lh), _r(rh), start=(i == 0), stop=(i == n - 1))


# ---------------------------------------------------------------------------
# Device program
# ---------------------------------------------------------------------------
def _build_program():
    _apply_tile_patch()
    nc = bass.Bass("TRN2", target_bir_lowering=False, debug=False, num_devices=8)

    def din(name, shape, dt=F32R):
        return nc.dram_tensor(name, list(shape), dt, kind="ExternalInput").ap()

    xT = din("xT", (C, SEQ), dt=BF16)
    xqT = din("xqT", (C, T), dt=BF16)
    wqk = din("wqk", (C, 2 * C), dt=BF16)  # q cols prescaled by 1/8
    wv = din("wv", (C, C), dt=BF16)
    wproj = din("wproj", (C, C))
    la_attn = din("la_attn", (C, RANK), dt=BF16)
    lb_qk = din("lb_qk", (RANK, 2 * C))  # q part zeroed for role 0, scaled
    lb_v = din("lb_v", (RANK, C))
    la_proj = din("la_proj", (C, RANK))
    lb_proj = din("lb_proj", (RANK, C))  # zeroed for role 0
    masks = din("masks", (NMASK, 128, 128), dt=BF16)
    ones1 = din("ones1", (1, 128))
    onescol = din("onescol", (128, 4), dt=BF16)
    yout = nc.dram_tensor("yout", [T, C], F32, kind="ExternalOutput").ap()

    with TileContext(nc) as tc:
        ctx = contextlib.ExitStack()
        with ctx:
            ctx.enter_context(
                nc.allow_low_precision(reason="float32r is full-width fp32 storage")
            )
            # K^T / V spill space (DRAM pool tiles so deps are tracked)
            dpool = ctx.enter_context(tc.tile_pool(name="dram", bufs=1, space="DRAM"))
            ktd = dpool.tile([C, SEQ], BF16)
            vd = dpool.tile([SEQ, C], BF16)

            # --- persistent SBUF ---
            persist = ctx.enter_context(tc.tile_pool(name="persist", bufs=1))
            qT_sb = persist.tile([128, NCH, T], BF16)  # Q^T (prescaled)
            la_attn_sb = persist.tile([128, NCH, RANK], BF16)
            lb_qk_sb = persist.tile([RANK, 2 * C], F32R)
            lb_v_sb = persist.tile([RANK, C], F32R)
            la_proj_sb = persist.tile([128, NCH, RANK], F32R)
            lb_proj_sb = persist.tile([RANK, C], F32R)
            ones1_sb = persist.tile([1, 128], F32R)
            onescol_sb = persist.tile([128, 4, 1], BF16)
            tmp_kv_sb = persist.tile([RANK, T], F32R)  # attn-lora mid, e2 rows

            nc.sync.dma_start(
                out=la_attn_sb[:], in_=la_attn.rearrange("(ch p) r -> p ch r", p=128)
            )
            nc.sync.dma_start(out=lb_qk_sb[:], in_=lb_qk[:])
            nc.sync.dma_start(out=lb_v_sb[:], in_=lb_v[:])
            nc.sync.dma_start(
                out=la_proj_sb[:], in_=la_proj.rearrange("(ch p) r -> p ch r", p=128)
            )
            nc.sync.dma_start(out=lb_proj_sb[:], in_=lb_proj[:])
            nc.sync.dma_start(out=ones1_sb[:], in_=ones1[:])
            nc.sync.dma_start(
                out=onescol_sb[:], in_=onescol.rearrange("p (h o) -> p h o", o=1)
            )

            # --- PSUM pools (3 + 5 = 8 banks) ---
            ps_s = ctx.enter_context(tc.tile_pool(name="ps_s", bufs=3, space="PSUM"))
            ps_y = ctx.enter_context(tc.tile_pool(name="ps_y", bufs=5, space="PSUM"))
            ps_misc = ps_s

            stage = ctx.enter_context(tc.tile_pool(name="stage", bufs=4))
            small = ctx.enter_context(tc.tile_pool(name="small", bufs=5))

            # ====== Phase A: K^T & V (spill) and Q^T (resident), one xT pass ======
            with tc.tile_pool(name="wqk_pool", bufs=1) as wqk_pool, tc.tile_pool(
                name="wv_pool", bufs=1
            ) as wv_pool, tc.tile_pool(name="xa", bufs=2) as xa_pool:
                wqk_sb = wqk_pool.tile([128, NCH, 2 * C], BF16)
                wv_sb = wv_pool.tile([128, NCH, C], BF16)
                for ch in range(NCH):
                    nc.sync.dma_start(
                        out=wqk_sb[:, ch, 0:C],
                        in_=wqk[128 * ch : 128 * (ch + 1), 0:C],
                    )
                for ch in range(NCH):
                    nc.sync.dma_start(
                        out=wqk_sb[:, ch, C:],
                        in_=wqk[128 * ch : 128 * (ch + 1), C:],
                    )
                for ch in range(NCH):
                    nc.sync.dma_start(
                        out=wv_sb[:, ch, :],
                        in_=wv[128 * ch : 128 * (ch + 1), :],
                    )

                for s in (0, 1):  # first seq blocks early (cheap PE start)
                    sl = slice(s * 512, (s + 1) * 512)
                    xt_s = xa_pool.tile([128, NCH, 512], BF16, tag="xa")
                    for ch in range(NCH):
                        nc.sync.dma_start(
                            out=xt_s[:, ch, :],
                            in_=xT[128 * ch : 128 * (ch + 1), sl],
                        )
                    if s >= 2:  # e2 rows: attn lora mid  tmp^T = A^T x
                        tsl = slice((s - 2) * 512, (s - 1) * 512)
                        tmp_ps = ps_misc.tile([RANK, 512], F32, tag="s")
                        _accum(
                            nc,
                            tmp_ps[:],
                            [
                                (la_attn_sb[:, ch, :], xt_s[:, ch, :])
                                for ch in range(NCH)
                            ],
                        )
                        nc.vector.tensor_copy(tmp_kv_sb[:, tsl], tmp_ps[:])
                    for m in range(NCH):  # kcol tiles
                        cols = slice(C + 128 * m, C + 128 * (m + 1))
                        kps = ps_s.tile([128, 512], F32, tag="s")
                        mms = [
                            (wqk_sb[:, ch, cols], xt_s[:, ch, :]) for ch in range(NCH)
                        ]
                        if s >= 2:
                            mms.append((lb_qk_sb[:, cols], tmp_kv_sb[:, tsl]))
                        _accum(nc, kps[:], mms)
                        kst = stage.tile([128, 512], BF16, tag="kvstage")
                        nc.vector.tensor_copy(kst[:], kps[:])
                        nc.sync.dma_start(
                            out=ktd[128 * m : 128 * (m + 1), sl], in_=kst[:]
                        )
                    for st in range(4):  # V: 128-row seq tiles within block
                        for vc in range(2):  # vcol halves
                            vcs = slice(512 * vc, 512 * (vc + 1))
                            vps = ps_s.tile([128, 512], F32, tag="s")
                            mms = [
                                (
                                    xt_s[:, ch, 128 * st : 128 * (st + 1)],
                                    wv_sb[:, ch, vcs],
                                )
                                for ch in range(NCH)
                            ]
                            if s >= 2:
                                base = (s - 2) * 512 + 128 * st
                                mms.append(
                                    (tmp_kv_sb[:, base : base + 128], lb_v_sb[:, vcs])
                                )
                            _accum(nc, vps[:], mms)
                            vst = stage.tile([128, 512], BF16, tag="kvstage")
                            nc.vector.tensor_copy(vst[:], vps[:])
                            row = s * 512 + st * 128
                            nc.sync.dma_start(
                                out=vd[row : row + 128, vcs], in_=vst[:]
                            )

            # =========== Phase B + C: attention and output projection ======
                # Q^T for my T rows (wqk q-cols are prescaled by 1/8)
                for s in range(2):
                    sl = slice(s * 512, (s + 1) * 512)
                    xq_s = xa_pool.tile([128, NCH, 512], BF16, tag="xa")
                    for ch in range(NCH):
                        nc.sync.dma_start(
                            out=xq_s[:, ch, :],
                            in_=xqT[128 * ch : 128 * (ch + 1), sl],
                        )
                    tmq_ps = ps_misc.tile([RANK, 512], F32, tag="s")
                    _accum(
                        nc,
                        tmq_ps[:],
                        [(la_attn_sb[:, ch, :], xq_s[:, ch, :]) for ch in range(NCH)],
                    )
                    tmq_sb = small.tile([RANK, 512], F32R, tag="tmq")
                    nc.vector.tensor_copy(tmq_sb[:], tmq_ps[:])
                    for m in range(NCH):
                        cols = slice(128 * m, 128 * (m + 1))
                        qps = ps_s.tile([128, 512], F32, tag="s")
                        mms = [
                            (wqk_sb[:, ch, cols], xq_s[:, ch, :]) for ch in range(NCH)
                        ]
                        mms.append((lb_qk_sb[:, cols], tmq_sb[:]))
                        _accum(nc, qps[:], mms)
                        nc.vector.tensor_copy(qT_sb[:, m, sl], qps[:])


                mask_sb = persist.tile([128, NMASK, 128], BF16)
                nc.sync.dma_start(out=mask_sb[:], in_=masks.rearrange("t p q -> p t q"))

                for s in (2, 3):  # remaining seq blocks
                    sl = slice(s * 512, (s + 1) * 512)
                    xt_s = xa_pool.tile([128, NCH, 512], BF16, tag="xa")
                    for ch in range(NCH):
                        nc.sync.dma_start(
                            out=xt_s[:, ch, :],
                            in_=xT[128 * ch : 128 * (ch + 1), sl],
                        )
                    if s >= 2:  # e2 rows: attn lora mid  tmp^T = A^T x
                        tsl = slice((s - 2) * 512, (s - 1) * 512)
                        tmp_ps = ps_misc.tile([RANK, 512], F32, tag="s")
                        _accum(
                            nc,
                            tmp_ps[:],
                            [
                                (la_attn_sb[:, ch, :], xt_s[:, ch, :])
                                for ch in range(NCH)
                            ],
                        )
                        nc.vector.tensor_copy(tmp_kv_sb[:, tsl], tmp_ps[:])
                    for m in range(NCH):  # kcol tiles
                        cols = slice(C + 128 * m, C + 128 * (m + 1))
                        kps = ps_s.tile([128, 512], F32, tag="s")
                        mms = [
                            (wqk_sb[:, ch, cols], xt_s[:, ch, :]) for ch in range(NCH)
                        ]
                        if s >= 2:
                            mms.append((lb_qk_sb[:, cols], tmp_kv_sb[:, tsl]))
                        _accum(nc, kps[:], mms)
                        kst = stage.tile([128, 512], BF16, tag="kvstage")
                        nc.vector.tensor_copy(kst[:], kps[:])
                        nc.sync.dma_start(
                            out=ktd[128 * m : 128 * (m + 1), sl], in_=kst[:]
                        )
                    for st in range(4):  # V: 128-row seq tiles within block
                        for vc in range(2):  # vcol halves
                            vcs = slice(512 * vc, 512 * (vc + 1))
                            vps = ps_s.tile([128, 512], F32, tag="s")
                            mms = [
                                (
                                    xt_s[:, ch, 128 * st : 128 * (st + 1)],
                                    wv_sb[:, ch, vcs],
                                )
                                for ch in range(NCH)
                            ]
                            if s >= 2:
                                base = (s - 2) * 512 + 128 * st
                                mms.append(
                                    (tmp_kv_sb[:, base : base + 128], lb_v_sb[:, vcs])
                                )
                            _accum(nc, vps[:], mms)
                            vst = stage.tile([128, 512], BF16, tag="kvstage")
                            nc.vector.tensor_copy(vst[:], vps[:])
                            row = s * 512 + st * 128
                            nc.sync.dma_start(
                                out=vd[row : row + 128, vcs], in_=vst[:]
                            )

            # =========== Phase B + C: attention and output projection ======
            bpool = ctx.enter_context(tc.tile_pool(name="bpool", bufs=1))
            wproj_sb = bpool.tile([128, NCH, C], F32R)
            nc.sync.dma_start(
                out=wproj_sb[:], in_=wproj.rearrange("(ch p) n -> p ch n", p=128)
            )
            y_acc = bpool.tile([128, NCH, T], F32R)  # pair p: heads 2p/2p+1

            kv_pool = ctx.enter_context(tc.tile_pool(name="kv", bufs=16))
            pt_pool = ctx.enter_context(tc.tile_pool(name="pt", bufs=10))

            def _emit_division(dqb, dpg, dyus):
                dqb_sl = slice(dqb * 512, (dqb + 1) * 512)
                for pi in range(2):
                    p = 2 * dpg + pi
                    for hi in range(2):
                        yu = dyus[2 * pi + hi]
                        ysb = pt_pool.tile(
                            [D + 1, 512], F32R, tag="ysb",
                            name=f"ysb_{dqb}_{dpg}_{pi}_{hi}",
                        )
                        nc.vector.tensor_copy(ysb[:], yu[:])
                        r_tmp = small.tile([1, 512], F32R, tag="rtmp")
                        nc.vector.reciprocal(r_tmp[:], ysb[D : D + 1, :])
                        r_bc = ps_y.tile([128, 512], F32, tag="y", name=f"rbc_{dqb}_{dpg}_{pi}_{hi}")
                        nc.tensor.matmul(
                            r_bc[:], ones1_sb[:], r_tmp[:], start=True, stop=True
                        )
                        rows = slice(64 * hi, 64 * hi + 64)
                        nc.vector.tensor_mul(
                            y_acc[rows, p, dqb_sl], ysb[0:D, :], r_bc[rows, :]
                        )

            pending = []
            for qb in range(2):
                qb_sl = slice(qb * 512, (qb + 1) * 512)
                ktl = _ktiles_for_qblock(qb)
                for pg in range(4):  # pair groups of 2 pairs (4 heads)
                    yus = {}
                    for hl in range(4):
                        yus[hl] = ps_y.tile([D + 1, 512], F32, tag="y", name=f"yu_{qb}_{pg}_{hl}")
                    for ki, (region, j) in enumerate(ktl):
                        qts = _active_qts(region, j, qb)
                        qlo, qw = qts[0], len(qts)
                        q_sl = slice(128 * qlo, 128 * (qlo + qw))
                        rel_sl = slice(128 * (qlo - 4 * qb), 128 * (qlo - 4 * qb + qw))
                        nq = 128 * qw
                        kbase = (0 if region == "e1" else T) + 128 * j
                        kt_s = kv_pool.tile([128, 2, 128], BF16, tag="kt")
                        nc.sync.dma_start(
                            out=kt_s[:],
                            in_=ktd[256 * pg : 256 * (pg + 1), kbase : kbase + 128]
                            .rearrange("(c p) n -> p c n", p=128),
                        )
                        va_s = kv_pool.tile([128, 4, D + 1], BF16, tag="va")
                        nc.sync.dma_start(
                            out=va_s[:, :, 0:D],
                            in_=vd[kbase : kbase + 128, 256 * pg : 256 * (pg + 1)]
                            .rearrange("p (h d) -> p h d", h=4),
                        )
                        nc.vector.tensor_copy(va_s[:, :, D : D + 1], onescol_sb[:])

                        if ki == 1 and pending:
                            _emit_division(*pending.pop(0))
                        for pi in range(2):
                            p = 2 * pg + pi
                            s_ps = []
                            for hi in range(2):
                                sp = ps_s.tile([128, 512], F32, tag="s")
                                lo = 64 * hi
                                nc.tensor.matmul(
                                    sp[:, 0:nq],
                                    kt_s[lo : lo + 64, pi, :],
                                    qT_sb[lo : lo + 64, p, q_sl],
                                    start=True,
                                    stop=True,
                                )
                                s_ps.append(sp)
                            for hi in range(2):
                                pt = pt_pool.tile([128, 512], BF16, tag="pt")
                                nc.scalar.activation(
                                    pt[:, 0:nq],
                                    s_ps[hi][:, 0:nq],
                                    mybir.ActivationFunctionType.Exp,
                                )
                                for qt in qts:
                                    if (region, j, qt) in MASK_IDX:
                                        mi = MASK_IDX[(region, j, qt)]
                                        rel = slice(
                                            128 * (qt - qlo), 128 * (qt - qlo + 1)
                                        )
                                        nc.vector.tensor_mul(
                                            pt[:, rel], pt[:, rel], mask_sb[:, mi, :]
                                        )
                                yu = yus[2 * pi + hi]
                                nc.tensor.matmul(
                                    yu[:, rel_sl],
                                    va_s[:, 2 * pi + hi, :],
                                    pt[:, 0:nq],
                                    start=(ki == 0),
                                    stop=(ki == len(ktl) - 1),
                                    skip_group_check=True,
                                )

                    pending.append((qb, pg, yus))

                while pending:
                    _emit_division(*pending.pop(0))

                # ---- Phase C: projection for this q-block ----
                tm2_ps = ps_misc.tile([RANK, 512], F32, tag="s")
                _accum(
                    nc,
                    tm2_ps[:],
                    [(la_proj_sb[:, ch, :], y_acc[:, ch, qb_sl]) for ch in range(NCH)],
                )
                tm2_sb = small.tile([RANK, 512], F32R, tag="tm2")
                nc.vector.tensor_copy(tm2_sb[:], tm2_ps[:])
                for qs in range(4):
                    qrow = 512 * qb + 128 * qs
                    for co in range(2):
                        cos = slice(512 * co, 512 * (co + 1))
                        ops = ps_s.tile([128, 512], F32, tag="s")
                        mms = [
                            (y_acc[:, ch, qrow : qrow + 128], wproj_sb[:, ch, cos])
                            for ch in range(NCH)
                        ]
                        mms.append(
                            (tm2_sb[:, 128 * qs : 128 * (qs + 1)], lb_proj_sb[:, cos])
                        )
                        _accum(nc, ops[:], mms)
                        ost = stage.tile([128, 512], F32, tag="stage")
                        nc.vector.tensor_copy(ost[:], ops[:])
                        nc.sync.dma_start(
                            out=yout[qrow : qrow + 128, cos], in_=ost[:]
                        )
    return nc


_PROGRAM = None


def _get_program():
    global _PROGRAM
    if _PROGRAM is None:
        _PROGRAM = _build_program()
    return _PROGRAM


# ---------------------------------------------------------------------------
# Host side
# ---------------------------------------------------------------------------
def _delayed_mask_np(t):
    ones = np.ones((t, t), dtype=bool)
    m11 = np.tril(ones) & np.triu(ones, -(LOOKAHEAD + OVERLAP))
    m12 = np.tril(ones, -LOOKAHEAD)
    m21 = np.tril(ones, LOOKAHEAD) & np.triu(ones, -OVERLAP)
    m22 = np.tril(ones)
    return np.block([[m11, m12], [m21, m22]])


def _core_inputs(core, e1, e2, W_attn, W_proj, la_attn, lb_attn, la_proj, lb_proj, M):
    b, r = core // 2, core % 2
    f32 = np.float32
    x = np.concatenate([e1[b], e2[b]], axis=0)  # [2T, C]
    xT = np.ascontiguousarray(x.T).astype(ml_dtypes.bfloat16)
    xq = e1[b] if r == 0 else e2[b]
    xqT = np.ascontiguousarray(xq.T).astype(ml_dtypes.bfloat16)

    wqk = np.array(W_attn[:, : 2 * C], dtype=f32)
    wqk[:, :C] *= QSCALE
    wqk = wqk.astype(ml_dtypes.bfloat16)
    lb_qk = np.array(lb_attn[:, : 2 * C], dtype=f32) * LSCALE
    lb_qk[:, :C] *= QSCALE
    if r == 0:
        lb_qk[:, :C] = 0.0
    lb_v = np.ascontiguousarray(lb_attn[:, 2 * C :], dtype=f32) * LSCALE
    lbp = np.array(lb_proj, dtype=f32) * LSCALE
    if r == 0:
        lbp[:] = 0.0

    masks = np.empty((NMASK, 128, 128), dtype=ml_dtypes.bfloat16)
    for i, (region, j, qt) in enumerate(MASK_TILES):
        qg = r * T + 128 * qt
        kg = (0 if region == "e1" else T) + 128 * j
        masks[i] = M[qg : qg + 128, kg : kg + 128].T.astype(f32)

    return {
        "xT": xT,
        "xqT": xqT,
        "wqk": wqk,
        "wv": np.ascontiguousarray(W_attn[:, 2 * C :]).astype(ml_dtypes.bfloat16),
        "wproj": np.ascontiguousarray(W_proj, dtype=f32),
        "la_attn": np.ascontiguousarray(la_attn).astype(ml_dtypes.bfloat16),
        "lb_qk": lb_qk,
        "lb_v": lb_v,
        "la_proj": np.ascontiguousarray(la_proj, dtype=f32),
        "lb_proj": lbp,
        "masks": masks,
        "ones1": np.ones((1, 128), dtype=f32),
        "onescol": np.ones((128, 4), dtype=ml_dtypes.bfloat16),
    }


def kernel(
    e1,
    e2,
    W_attn,
    W_proj,
    lora_A_attn,
    lora_B_attn,
    lora_A_proj,
    lora_B_proj,
    _trace=False,
):
    e1 = np.asarray(e1, np.float32)
    e2 = np.asarray(e2, np.float32)
    nc = _get_program()
    M = _delayed_mask_np(T)
    in_maps = [
        _core_inputs(
            c, e1, e2, W_attn, W_proj, lora_A_attn, lora_B_attn, lora_A_proj,
            lora_B_proj, M,
        )
        for c in range(8)
    ]
    res = run_bass_kernel_spmd(nc, in_maps, core_ids=list(range(8)), trace=_trace)
    y1 = np.stack([res.results[2 * b]["yout"] for b in range(B)])
    y2 = np.stack([res.results[2 * b + 1]["yout"] for b in range(B)])
    if _trace:
        kernel.last_results = res
    return y1, y2



# revision 42
# speedup vs baseline: 1.2125x; 1.2125x over previous
"""Trainium2 Bass kernel for nn_DelayedSelfAttention (B=4, T=1024, C=1024, H=16).

Sharding: 8 cores = 4 batches x 2 sequence-halves.  Core c handles batch
c//2 and query rows [r*T, (r+1)*T) of the concatenated [2T] sequence
(r = c%2).  Each core computes K/V for the full 2T sequence (duplicated
kv-projection -- cheaper than any collective on this fabric), attention
for its T query rows over all 16 heads, and the output projection for
its rows.  Role asymmetry (mask values, q/proj LoRA) is pushed into
per-core input data so a single SPMD program serves all cores.

v2 vs the spill-to-DRAM baseline:
 - K^T and V stay RESIDENT in SBUF (no DRAM spill + reload).
 - QKV projections run as compensated fp8e4m3 DoubleRow matmuls:
   x ~ x8 + xlo, W ~ W8 + Wlo (host-quantized; weights prescaled by 64
   to clear the e4m3 subnormal range, staging copies scale by 1/64).
   Three DR terms (x8W8 + xloW8 + x8Wlo) cover a 256-deep contraction
   in 1.5 row-passes vs bf16's 2 -- ~25% tensor-engine saving at ~0.25%
   error (compensation cancels first-order quantization).
 - exp batched per head-pair ([128, 2, nq] PSUM duos), masks multiplied
   with a stride-0 head-broadcast, head-phase staging copies on the
   (otherwise idle) scalar engine.
 - emission order software-pipelines the phases: Q-proj, K/V blocks
   s0..s2, q-block-0 attention overlapping the s3 projection, then the
   qb0 output projection, qb1 attention, qb1 projection.
"""

import contextlib
import sys

for _p in ("/opt/trn_rl_repo", "/root/.axon_site/_ro/trn_rl_repo"):
    if _p not in sys.path:
        sys.path.insert(0, _p)

import ml_dtypes
import numpy as np

import concourse.bass as bass
import concourse.mybir as mybir
import concourse.tile as tile_mod
from concourse.bass_utils import run_bass_kernel_spmd
from concourse.tile import TileContext
from concourse.vector_clock import ScopedClock

# ---------------------------------------------------------------------------
# Workaround: this walrus build supports a single semaphore wait per
# instruction.  Split multi-wait instructions into same-engine NoOps each
# carrying one wait (identical sequencer semantics).
# ---------------------------------------------------------------------------
_ws_counter = [0]


def _fresh_name():
    _ws_counter[0] += 1
    return f"I-waitsplit-{_ws_counter[0]}"


def _split_inst_waits(inst):
    si = inst.sync_info
    if si is None:
        return []
    waits = list(si.on_wait or [])
    if len(waits) <= 1:
        return []
    nops = []
    for w in waits[:-1]:
        nop = mybir.InstNoOp(name=_fresh_name())
        nop.engine = inst.engine
        nop.sync_info = mybir.SyncInfo(on_wait=[w], on_update=[])
        nops.append(nop)
    inst.sync_info = mybir.SyncInfo(
        on_wait=[waits[-1]], on_update=list(si.on_update or [])
    )
    return nops


_orig_lower = tile_mod.TileContext._lower_ordered_insts


def _patched_lower(self, ordered):
    for bb_name in list(ordered.keys()):
        new = []
        for inst in ordered[bb_name]:
            new.extend(_split_inst_waits(inst))
            new.append(inst)
        ordered[bb_name] = new
    return _orig_lower(self, ordered)


def _patched_drain_and_barrier(self, tick_clock, wait_clock):
    nc = self.nc
    drain_inst = nc.sync.drain()
    wait_clock.add_sem_waits(
        drain_inst.ins, ScopedClock({None: tick_clock.global_clock})
    )
    nops = _split_inst_waits(drain_inst.ins)
    if nops:
        first_wait = drain_inst.ins.sync_info
        drain_inst.ins.sync_info = mybir.SyncInfo(on_wait=[], on_update=[])
        for nop in nops:
            n2 = nc.sync.nop(nofuse=True)
            n2.ins.sync_info = nop.sync_info
        d2 = nc.sync.drain()
        d2.ins.sync_info = first_wait

    nc.all_engine_barrier()
    assert self.sems is not None
    popped = nc._tile_sem_poison_stack.pop()
    assert popped is self._sem_poison
    nc.clear_and_free_semaphores(list(self.sems.allocated().values()))
    nc.all_engine_barrier()


def _apply_tile_patch():
    if tile_mod.TileContext._lower_ordered_insts is not _patched_lower:
        tile_mod.TileContext._lower_ordered_insts = _patched_lower
        tile_mod.TileContext._drain_and_barrier = _patched_drain_and_barrier


# ---------------------------------------------------------------------------
# Problem constants (hardcoded per the task contract).
# ---------------------------------------------------------------------------
B, T, C, H = 4, 1024, 1024, 16
D = C // H  # 64
SEQ = 2 * T
LOOKAHEAD, OVERLAP = 64, 64
RANK, ALPHA = 8, 16.0
RPAD = 16  # lora-A stationary padded (dual-fp8 ldweights needs width >= 16)
LSCALE = ALPHA / RANK  # 2.0
QSCALE = 1.0 / np.sqrt(D)  # 1/8
WSC = 64.0  # fp8 weight prescale (cleared by 1/WSC at staging)
NCH = C // 128  # 8 c-chunks
NCP = NCH // 2  # 4 c-chunk-pairs (DoubleRow)
NQT = T // 128  # 8 q-subtiles per core
F32 = mybir.dt.float32
F32R = mybir.dt.float32r
BF16 = mybir.dt.bfloat16
F8E4 = mybir.dt.float8e4
FP8NP = ml_dtypes.float8_e4m3fn
DR = mybir.MatmulPerfMode.DoubleRow


# Trace-time tiling structure, shared by host (mask packing) and device.
def _ktiles_for_qblock(qb):
    """k-tiles (region, j) touched by q-subtiles [4qb, 4qb+4)."""
    qts = range(4 * qb, 4 * qb + 4)
    e1 = sorted({j for qt in qts for j in (qt - 1, qt, qt + 1) if 0 <= j < NQT})
    e2 = sorted({j for qt in qts for j in range(qt + 1)})
    return [("e1", j) for j in e1] + [("e2", j) for j in e2]


def _active_qts(region, j, qb):
    if region == "e1":
        qts = [qt for qt in range(4 * qb, 4 * qb + 4) if j in (qt - 1, qt, qt + 1)]
    else:
        qts = [qt for qt in range(4 * qb, 4 * qb + 4) if j <= qt]
    assert qts == list(range(qts[0], qts[-1] + 1))
    return qts


def _mask_tiles():
    out = []
    for qt in range(NQT):
        for j in (qt - 1, qt, qt + 1):
            if 0 <= j < NQT:
                out.append(("e1", j, qt))
        for j in (qt - 1, qt):
            if j >= 0:
                out.append(("e2", j, qt))
    return out


MASK_TILES = _mask_tiles()  # 37 tiles
MASK_IDX = {k: i for i, k in enumerate(MASK_TILES)}
NMASK = len(MASK_TILES)


# ---------------------------------------------------------------------------
# Device program
# ---------------------------------------------------------------------------
def _build_program():
    _apply_tile_patch()
    nc = bass.Bass("TRN2", target_bir_lowering=False, debug=False, num_devices=8)

    def din(name, shape, dt=F32R):
        return nc.dram_tensor(name, list(shape), dt, kind="ExternalInput").ap()

    x8 = din("x8", (128, NCP, 2, SEQ), dt=F8E4)
    xlo = din("xlo", (128, NCP, 2, SEQ), dt=F8E4)
    xq8 = din("xq8", (128, NCP, 2, T), dt=F8E4)
    xqlo = din("xqlo", (128, NCP, 2, T), dt=F8E4)
    wq8 = din("wq8", (128, NCP, 2, C), dt=F8E4)
    wqlo = din("wqlo", (128, NCP, 2, C), dt=F8E4)
    wk8 = din("wk8", (128, NCP, 2, C), dt=F8E4)
    wklo = din("wklo", (128, NCP, 2, C), dt=F8E4)
    wv8 = din("wv8", (128, NCP, 2, C), dt=F8E4)
    wvlo = din("wvlo", (128, NCP, 2, C), dt=F8E4)
    la8 = din("la8", (128, NCP, 2, RPAD), dt=F8E4)
    lalo = din("lalo", (128, NCP, 2, RPAD), dt=F8E4)
    lb_qk = din("lb_qk", (RANK, 2 * C), dt=BF16)  # scaled, role-zeroed q
    lb_v = din("lb_v", (RANK, C), dt=BF16)
    la_proj = din("la_proj", (128, NCH, RANK), dt=BF16)
    lb_proj = din("lb_proj", (RANK, C), dt=BF16)  # zeroed for role 0
    wproj = din("wproj", (128, NCH, C), dt=BF16)
    masks = din("masks", (NMASK, 128, 128), dt=BF16)
    ones1 = din("ones1", (1, 128))
    yout = nc.dram_tensor("yout", [T, C], F32, kind="ExternalOutput").ap()

    with TileContext(nc) as tc:
        ctx = contextlib.ExitStack()
        with ctx:
            ctx.enter_context(
                nc.allow_low_precision(reason="float32r is full-width fp32 storage")
            )
            # --- persistent SBUF ---
            persist = ctx.enter_context(tc.tile_pool(name="persist", bufs=1))
            ktsb = persist.tile([128, NCH, SEQ], BF16)      # resident K^T
            vres = persist.tile([128, 16, H, D + 1], BF16)  # resident V + ones col
            qT_sb = persist.tile([128, NCH, T], BF16)       # resident Q^T (prescaled)
            y_acc = persist.tile([128, NCH, T], BF16)       # normalized y
            mask_sb = persist.tile([128, NMASK, 128], BF16)
            tmp_kv = persist.tile([RANK, T], BF16)          # e2 attn-lora mid
            la8_sb = persist.tile([128, NCP, 2, RPAD], F8E4)
            lalo_sb = persist.tile([128, NCP, 2, RPAD], F8E4)
            lb_qk_sb = persist.tile([RANK, 2 * C], BF16)
            lb_v_sb = persist.tile([RANK, C], BF16)
            la_proj_sb = persist.tile([128, NCH, RANK], BF16)
            lb_proj_sb = persist.tile([RANK, C], BF16)
            ones1_sb = persist.tile([1, 128], F32R)

            nc.vector.memset(vres[:, :, :, D : D + 1], 1.0)  # ones column

            # --- PSUM pools: 4 (yu / r_bc) + 2x2 (score duos) = 8 banks
            ps_y = ctx.enter_context(tc.tile_pool(name="ps_y", bufs=4, space="PSUM"))
            ps_s = ctx.enter_context(tc.tile_pool(name="ps_s", bufs=2, space="PSUM"))

            small = ctx.enter_context(tc.tile_pool(name="small", bufs=1))
            pt_pool = ctx.enter_context(tc.tile_pool(name="pt", bufs=10))
            ysb_pool = ctx.enter_context(tc.tile_pool(name="ysb", bufs=3))

            pending = []
            holders = {}

            def _division_stage1(dyus):
                ysbs = []
                for hi in range(2):
                    ysb = ysb_pool.tile([D + 1, 512], F32R, tag="ysb")
                    nc.vector.tensor_copy(ysb[:], dyus[hi][:])
                    r_tmp = ysb_pool.tile([1, 512], F32R, tag="rt")
                    nc.vector.reciprocal(r_tmp[:], ysb[D : D + 1, :])
                    ysbs.append((ysb, r_tmp))
                return ysbs

            def _division_stage2(dqb, dp, ysbs):
                dqb_sl = slice(dqb * 512, (dqb + 1) * 512)
                y_acc = holders["y_acc"]
                for hi in range(2):
                    ysb, r_tmp = ysbs[hi]
                    r_bc = ps_y.tile([128, 512], F32, tag="y")
                    nc.tensor.matmul(
                        r_bc[:], ones1_sb[:], r_tmp[:],
                        start=True, stop=True,
                    )
                    rows = slice(64 * hi, 64 * hi + 64)
                    nc.vector.tensor_mul(
                        y_acc[rows, dp, dqb_sl], ysb[0:D, :], r_bc[rows, :]
                    )

            def emit_attention_p(qb, p, bracket=()):
                """Attention for q-block qb, c-chunk p (head pair 2p, 2p+1).

                AV matmuls lag scores by one k-tile so the tensor engine has
                work while exp/mask chains drain; bracket thunks (projection
                work) are popped one per k-tile to fill remaining gaps.
                """
                items = list(bracket)
                ktl = _ktiles_for_qblock(qb)
                yus = [
                    ps_y.tile([D + 1, 512], F32, tag="y", name=f"yu_{qb}_{p}_{i}")
                    for i in range(2)
                ]
                avq = []

                def flush_av():
                    ki0, pt0, nq0, rel0, st0 = avq.pop(0)
                    for hi in range(2):
                        nc.tensor.matmul(
                            yus[hi][:, rel0],
                            vres[:, st0, 2 * p + hi, :],
                            pt0[:, hi, 0:nq0],
                            start=(ki0 == 0),
                            stop=(ki0 == len(ktl) - 1),
                            skip_group_check=True,
                        )

                for ki, (region, j) in enumerate(ktl):
                    qts = _active_qts(region, j, qb)
                    qlo, qw = qts[0], len(qts)
                    q_sl = slice(128 * qlo, 128 * (qlo + qw))
                    rel_sl = slice(128 * (qlo - 4 * qb), 128 * (qlo - 4 * qb + qw))
                    nq = 128 * qw
                    kbase = (0 if region == "e1" else T) + 128 * j
                    st_glob = kbase // 128

                    if ki == 1 and pending:
                        _division_stage2(*pending.pop(0))
                    if items and ki >= 1:
                        items.pop(0)()

                    sp = ps_s.tile([128, 2, 512], F32, tag="s")
                    for hi in range(2):
                        lo = 64 * hi
                        nc.tensor.matmul(
                            sp[:, hi, 0:nq],
                            ktsb[lo : lo + 64, p, kbase : kbase + 128],
                            qT_sb[lo : lo + 64, p, q_sl],
                            start=True,
                            stop=True,
                        )
                    pt = pt_pool.tile([128, 2, 512], BF16, tag="pt")
                    nc.scalar.activation(
                        pt[:, :, 0:nq],
                        sp[:, :, 0:nq],
                        mybir.ActivationFunctionType.Exp,
                    )
                    for qt in qts:
                        if (region, j, qt) in MASK_IDX:
                            mi = MASK_IDX[(region, j, qt)]
                            rel = slice(128 * (qt - qlo), 128 * (qt - qlo + 1))
                            mb = mask_sb[:, mi : mi + 1, :].broadcast_to(
                                [128, 2, 128]
                            )
                            nc.vector.tensor_mul(pt[:, :, rel], pt[:, :, rel], mb)
                    avq.append((ki, pt, nq, rel_sl, st_glob))
                    flush_av()
                while avq:
                    flush_av()
                for it in items:
                    it()
                pending.append((qb, p, _division_stage1(yus)))

            # ===== phase A: projections (compensated fp8 DoubleRow) ============
            def mid_group(out_ap, x8_t, xlo_t, sl):
                """attn-lora mid: sum_c A[c, :]^T x[c, sl] -> [RPAD, 512]."""
                i = 0
                for cp in range(NCP):
                    for lh, rh in (
                        (la8_sb, x8_t), (lalo_sb, x8_t), (la8_sb, xlo_t),
                    ):
                        nc.tensor.matmul(
                            out_ap,
                            lh[:, cp, :, :],
                            rh[:, cp, :, sl],
                            start=(i == 0),
                            stop=(i == 3 * NCP - 1),
                            perf_mode=DR,
                        )
                        i += 1

            actx = contextlib.ExitStack()
            with actx:
                wk_pool = actx.enter_context(tc.tile_pool(name="wk", bufs=1))
                xa_pool = actx.enter_context(tc.tile_pool(name="xa", bufs=2))

                def load_xq(s):
                    sl = slice(s * 512, (s + 1) * 512)
                    xq8_t = xa_pool.tile([128, NCP, 2, 512], F8E4, tag="x8",
                                         name=f"xq8_{s}")
                    xqlo_t = xa_pool.tile([128, NCP, 2, 512], F8E4, tag="xlo",
                                          name=f"xqlo_{s}")
                    nc.sync.dma_start(out=xq8_t[:], in_=xq8[:, :, :, sl])
                    nc.sync.dma_start(out=xqlo_t[:], in_=xqlo[:, :, :, sl])
                    return xq8_t, xqlo_t

                # ---- Q^T projection first (own T rows), resident ----
                with tc.tile_pool(name="wq", bufs=1) as wq_pool:
                    wq8_sb = wq_pool.tile([128, NCP, 2, C], F8E4)
                    wqlo_sb = wq_pool.tile([128, NCP, 2, C], F8E4)
                    nc.sync.dma_start(out=wq8_sb[:], in_=wq8[:])
                    xqt = {s_: load_xq(s_) for s_ in range(2)}
                    nc.sync.dma_start(out=la8_sb[:], in_=la8[:])
                    nc.sync.dma_start(out=lalo_sb[:], in_=lalo[:])
                    nc.sync.dma_start(out=wqlo_sb[:], in_=wqlo[:])
                    nc.sync.dma_start(out=lb_qk_sb[:], in_=lb_qk[:])
                    # prefetch K weights while Q computes
                    wk8_sb = wk_pool.tile([128, NCP, 2, C], F8E4)
                    wklo_sb = wk_pool.tile([128, NCP, 2, C], F8E4)
                    nc.sync.dma_start(out=wk8_sb[:], in_=wk8[:])
                    nc.sync.dma_start(out=wklo_sb[:], in_=wklo[:])
                    nc.sync.dma_start(
                        out=mask_sb[:], in_=masks.rearrange("t p q -> p t q")
                    )
                    nc.sync.dma_start(out=lb_v_sb[:], in_=lb_v[:])
                    nc.sync.dma_start(out=la_proj_sb[:], in_=la_proj[:])
                    nc.sync.dma_start(out=lb_proj_sb[:], in_=lb_proj[:])
                    nc.sync.dma_start(out=ones1_sb[:], in_=ones1[:])

                    for s_ in range(2):
                        sl = slice(s_ * 512, (s_ + 1) * 512)
                        xq8_t, xqlo_t = xqt[s_]
                        tmq_ps = ps_s.tile([128, 2, 512], F32, tag="s")
                        mid_group(tmq_ps[0:RPAD, 0, :], xq8_t, xqlo_t,
                                  slice(0, 512))
                        tmq_sb = small.tile([RANK, 512], BF16, tag="tmq")
                        nc.vector.tensor_scalar_mul(
                            tmq_sb[:], tmq_ps[0:RANK, 0, :], 1.0 / WSC
                        )
                        for mp in range(4):
                            qps = ps_s.tile([128, 2, 512], F32, tag="s")
                            for h2 in range(2):
                                m = 2 * mp + h2
                                cols = slice(128 * m, 128 * (m + 1))
                                i = 0
                                for lh, rh in (
                                    (wq8_sb, xq8_t),
                                    (wqlo_sb, xq8_t),
                                    (wq8_sb, xqlo_t),
                                ):
                                    for cp in range(NCP):
                                        nc.tensor.matmul(
                                            qps[:, h2, :],
                                            lh[:, cp, :, cols],
                                            rh[:, cp, :, :],
                                            start=(i == 0),
                                            stop=False,
                                            perf_mode=DR,
                                        )
                                        i += 1
                                nc.tensor.matmul(
                                    qps[:, h2, :],
                                    lb_qk_sb[:, cols],
                                    tmq_sb[:],
                                    start=False,
                                    stop=True,
                                )
                            nc.scalar.mul(
                                qT_sb[:, 2 * mp : 2 * mp + 2, sl], qps[:], 1.0 / WSC
                            )

                wv_pool = actx.enter_context(tc.tile_pool(name="wv", bufs=1))
                wv8_sb = wv_pool.tile([128, NCP, 2, C], F8E4)
                wvlo_sb = wv_pool.tile([128, NCP, 2, C], F8E4)
                nc.sync.dma_start(out=wv8_sb[:], in_=wv8[:])
                nc.sync.dma_start(out=wvlo_sb[:], in_=wvlo[:])
                holders["y_acc"] = y_acc

                def load_x(s):
                    sl = slice(s * 512, (s + 1) * 512)
                    x8_t = xa_pool.tile([128, NCP, 2, 512], F8E4, tag="x8",
                                        name=f"x8_{s}")
                    xlo_t = xa_pool.tile([128, NCP, 2, 512], F8E4, tag="xlo",
                                         name=f"xlo_{s}")
                    nc.sync.dma_start(out=x8_t[:], in_=x8[:, :, :, sl])
                    nc.sync.dma_start(out=xlo_t[:], in_=xlo[:, :, :, sl])
                    return x8_t, xlo_t

                def emit_mid(s, x8_t, xlo_t):
                    tsl = slice((s - 2) * 512, (s - 1) * 512)
                    tmp_ps = ps_s.tile([128, 2, 512], F32, tag="s")
                    mid_group(tmp_ps[0:RPAD, 0, :], x8_t, xlo_t, slice(0, 512))
                    nc.vector.tensor_scalar_mul(
                        tmp_kv[:, tsl], tmp_ps[0:RANK, 0, :], 1.0 / WSC
                    )

                def emit_k_duo(s, mp, x8_t, xlo_t, stage_on_act=True):
                    sl = slice(s * 512, (s + 1) * 512)
                    tsl = slice((s - 2) * 512, (s - 1) * 512) if s >= 2 else None
                    kps = ps_s.tile([128, 2, 512], F32, tag="s")
                    for h2 in range(2):
                        m = 2 * mp + h2
                        cols = slice(128 * m, 128 * (m + 1))
                        i = 0
                        for lh, rh in (
                            (wk8_sb, x8_t), (wklo_sb, x8_t), (wk8_sb, xlo_t),
                        ):
                            for cp in range(NCP):
                                nc.tensor.matmul(
                                    kps[:, h2, :],
                                    lh[:, cp, :, cols],
                                    rh[:, cp, :, :],
                                    start=(i == 0),
                                    stop=(i == 3 * NCP - 1 and s < 2),
                                    perf_mode=DR,
                                )
                                i += 1
                        if s >= 2:
                            nc.tensor.matmul(
                                kps[:, h2, :],
                                lb_qk_sb[:, C + 128 * m : C + 128 * (m + 1)],
                                tmp_kv[:, tsl],
                                start=False,
                                stop=True,
                            )
                    dst = ktsb[:, 2 * mp : 2 * mp + 2, sl]
                    if stage_on_act:
                        nc.scalar.mul(dst, kps[:], 1.0 / WSC)
                    else:
                        nc.vector.tensor_scalar_mul(dst, kps[:], 1.0 / WSC)

                def emit_v_duo(s, st, x8_t, xlo_t, stage_on_act=True):
                    ssl = slice(128 * st, 128 * (st + 1))
                    vps = ps_s.tile([128, 2, 512], F32, tag="s")
                    for vc in range(2):
                        vsl = slice(512 * vc, 512 * (vc + 1))
                        i = 0
                        for lh, rh in (
                            (x8_t, wv8_sb), (xlo_t, wv8_sb), (x8_t, wvlo_sb),
                        ):
                            for cp in range(NCP):
                                nc.tensor.matmul(
                                    vps[:, vc, :],
                                    lh[:, cp, :, ssl],
                                    rh[:, cp, :, vsl],
                                    start=(i == 0),
                                    stop=(i == 3 * NCP - 1 and s < 2),
                                    perf_mode=DR,
                                )
                                i += 1
                        if s >= 2:
                            base = (s - 2) * 512 + 128 * st
                            nc.tensor.matmul(
                                vps[:, vc, :],
                                tmp_kv[:, base : base + 128],
                                lb_v_sb[:, vsl],
                                start=False,
                                stop=True,
                            )
                    dst = vres[:, 4 * s + st, :, 0:D]
                    vsrc = vps[:].rearrange("p v (h d) -> p (v h) d", h=8)
                    if stage_on_act:
                        nc.scalar.mul(dst, vsrc, 1.0 / WSC)
                    else:
                        nc.vector.tensor_scalar_mul(dst, vsrc, 1.0 / WSC)

                def emit_v_single(s, st, vc, x8_t, xlo_t, stage_on_act=True):
                    ssl = slice(128 * st, 128 * (st + 1))
                    vsl = slice(512 * vc, 512 * (vc + 1))
                    vps = ps_s.tile([128, 2, 512], F32, tag="s")
                    i = 0
                    for lh, rh in (
                        (x8_t, wv8_sb), (xlo_t, wv8_sb), (x8_t, wvlo_sb),
                    ):
                        for cp in range(NCP):
                            nc.tensor.matmul(
                                vps[:, 0, :],
                                lh[:, cp, :, ssl],
                                rh[:, cp, :, vsl],
                                start=(i == 0),
                                stop=(i == 3 * NCP - 1 and s < 2),
                                perf_mode=DR,
                            )
                            i += 1
                    if s >= 2:
                        base = (s - 2) * 512 + 128 * st
                        nc.tensor.matmul(
                            vps[:, 0, :],
                            tmp_kv[:, base : base + 128],
                            lb_v_sb[:, vsl],
                            start=False,
                            stop=True,
                        )
                    dst = vres[:, 4 * s + st, 8 * vc : 8 * vc + 8, 0:D]
                    vsrc = vps[:, 0, :].rearrange("p (h d) -> p h d", h=8)
                    if stage_on_act:
                        nc.scalar.mul(dst, vsrc, 1.0 / WSC)
                    else:
                        nc.vector.tensor_scalar_mul(dst, vsrc, 1.0 / WSC)

                # ---- blocks s0..s2 up front; s3 is emitted inside qb0
                # attention windows to keep the tensor engine fed ----
                for s_ in range(3):
                    x8_t, xlo_t = load_x(s_)
                    if s_ >= 2:
                        emit_mid(s_, x8_t, xlo_t)
                    for mp in range(4):
                        emit_k_duo(s_, mp, x8_t, xlo_t)
                    for st in range(4):
                        emit_v_duo(s_, st, x8_t, xlo_t)

                s3x = {}

                def s3_item(kind, idx):
                    def run():
                        if "x" not in s3x:
                            s3x["x"] = load_x(3)
                            emit_mid(3, *s3x["x"])
                        x8_t, xlo_t = s3x["x"]
                        if kind == "k":
                            emit_k_duo(3, idx, x8_t, xlo_t, stage_on_act=False)
                        else:
                            emit_v_duo(3, idx, x8_t, xlo_t, stage_on_act=False)
                    return run

                brackets0 = {
                    1: [s3_item("k", 0), s3_item("v", 0)],
                    2: [s3_item("k", 1), s3_item("v", 1)],
                    3: [s3_item("k", 2), s3_item("v", 2)],
                    4: [s3_item("k", 3), s3_item("v", 3)],
                }
                for p in range(8):
                    emit_attention_p(0, p, bracket=brackets0.get(p, ()))

            # ===== phase B: output projection + qb1 ===========================
            bpool = ctx.enter_context(tc.tile_pool(name="bpool", bufs=1))
            wproj_sb = bpool.tile([128, NCH, C], BF16)
            nc.sync.dma_start(out=wproj_sb[:], in_=wproj[:])
            ost_pool = ctx.enter_context(tc.tile_pool(name="ost", bufs=2))

            proj_state = {}

            def proj_tm2(qb):
                qb_sl = slice(qb * 512, (qb + 1) * 512)
                tm2_ps = ps_s.tile([128, 2, 512], F32, tag="s")
                for ch in range(NCH):
                    nc.tensor.matmul(
                        tm2_ps[0:RANK, 0, :],
                        la_proj_sb[:, ch, :],
                        y_acc[:, ch, qb_sl],
                        start=(ch == 0),
                        stop=(ch == NCH - 1),
                    )
                tm2_sb = small.tile([RANK, 512], BF16, tag="tm2")
                nc.vector.tensor_copy(tm2_sb[:], tm2_ps[0:RANK, 0, :])
                proj_state[qb] = tm2_sb

            def proj_qs(qb, qs):
                tm2_sb = proj_state[qb]
                qrow = 512 * qb + 128 * qs
                ops = ps_s.tile([128, 2, 512], F32, tag="s")
                for co in range(2):
                    cos = slice(512 * co, 512 * (co + 1))
                    for ch in range(NCH):
                        nc.tensor.matmul(
                            ops[:, co, :],
                            y_acc[:, ch, qrow : qrow + 128],
                            wproj_sb[:, ch, cos],
                            start=(ch == 0),
                            stop=False,
                        )
                    nc.tensor.matmul(
                        ops[:, co, :],
                        tm2_sb[:, 128 * qs : 128 * (qs + 1)],
                        lb_proj_sb[:, cos],
                        start=False,
                        stop=True,
                    )
                ost = ost_pool.tile([128, 2, 512], F32, tag="ost")
                nc.vector.tensor_copy(ost[:], ops[:])
                nc.sync.dma_start(
                    out=yout[qrow : qrow + 128, :],
                    in_=ost[:].rearrange("p a b -> p (a b)"),
                )

            proj0 = [lambda: proj_tm2(0)] + [
                (lambda qs=qs: proj_qs(0, qs)) for qs in range(4)
            ]
            brackets1 = {1: proj0[0:2], 2: proj0[2:3], 3: proj0[3:4], 4: proj0[4:5]}
            for p in range(8):
                emit_attention_p(1, p, bracket=brackets1.get(p, ()))

            while pending:
                _division_stage2(*pending.pop(0))
            proj_tm2(1)
            for qs in range(4):
                proj_qs(1, qs)
    return nc.dram_tensor(name, list(shape), dt, kind="ExternalInput").ap()

    x8 = din("x8", (128, NCP, 2, SEQ), dt=F8E4)
    xlo = din("xlo", (128, NCP, 2, SEQ), dt=F8E4)
    xq8 = din("xq8", (128, NCP, 2, T), dt=F8E4)
    xqlo = din("xqlo", (128, NCP, 2, T), dt=F8E4)
    wq8 = din("wq8", (128, NCP, 2, C), dt=F8E4)
    wqlo = din("wqlo", (128, NCP, 2, C), dt=F8E4)
    wk8 = din("wk8", (128, NCP, 2, C), dt=F8E4)
    wklo = din("wklo", (128, NCP, 2, C), dt=F8E4)
    wv8 = din("wv8", (128, NCP, 2, C), dt=F8E4)
    wvlo = din("wvlo", (128, NCP, 2, C), dt=F8E4)
    la8 = din("la8", (128, NCP, 2, RPAD), dt=F8E4)
    lalo = din("lalo", (128, NCP, 2, RPAD), dt=F8E4)
    lb_qk = din("lb_qk", (RANK, 2 * C), dt=BF16)  # scaled, role-zeroed q
    lb_v = din("lb_v", (RANK, C), dt=BF16)
    la_proj = din("la_proj", (128, NCH, RANK), dt=BF16)
    lb_proj = din("lb_proj", (RANK, C), dt=BF16)  # zeroed for role 0
    wproj = din("wproj", (128, NCH, C), dt=BF16)
    masks = din("masks", (NMASK, 128, 128), dt=BF16)
    ones1 = din("ones1", (1, 128))
    yout = nc.dram_tensor("yout", [T, C], F32, kind="ExternalOutput").ap()

    with TileContext(nc) as tc:
        ctx = contextlib.ExitStack()
        with ctx:
            ctx.enter_context(
                nc.allow_low_precision(reason="float32r is full-width fp32 storage")
            )
            # --- persistent SBUF ---
            persist = ctx.enter_context(tc.tile_pool(name="persist", bufs=1))
            ktsb = persist.tile([128, NCH, SEQ], BF16)      # resident K^T
            vres = persist.tile([128, 16, H, D + 1], BF16)  # resident V + ones col
            qT_sb = persist.tile([128, NCH, T], BF16)       # resident Q^T (prescaled)
            y_acc = persist.tile([128, NCH, T], BF16)       # normalized y
            mask_sb = persist.tile([128, NMASK, 128], BF16)
            tmp_kv = persist.tile([RANK, T], BF16)          # e2 attn-lora mid
            la8_sb = persist.tile([128, NCP, 2, RPAD], F8E4)
            lalo_sb = persist.tile([128, NCP, 2, RPAD], F8E4)
            lb_qk_sb = persist.tile([RANK, 2 * C], BF16)
            lb_v_sb = persist.tile([RANK, C], BF16)
            la_proj_sb = persist.tile([128, NCH, RANK], BF16)
            lb_proj_sb = persist.tile([RANK, C], BF16)
            ones1_sb = persist.tile([1, 128], F32R)

            nc.vector.memset(vres[:, :, :, D : D + 1], 1.0)  # ones column

            # --- PSUM pools: 4 (yu / r_bc) + 2x2 (score duos) = 8 banks
            ps_y = ctx.enter_context(tc.tile_pool(name="ps_y", bufs=4, space="PSUM"))
            ps_s = ctx.enter_context(tc.tile_pool(name="ps_s", bufs=2, space="PSUM"))

            small = ctx.enter_context(tc.tile_pool(name="small", bufs=1))
            pt_pool = ctx.enter_context(tc.tile_pool(name="pt", bufs=10))
            ysb_pool = ctx.enter_context(tc.tile_pool(name="ysb", bufs=3))

            # ===== attention helpers (phase-A-pool-free) =======================
            pending = []

            def _emit_division(dqb, dp, dyus):
                dqb_sl = slice(dqb * 512, (dqb + 1) * 512)
                for hi in range(2):
                    yu = dyus[hi]
                    ysb = ysb_pool.tile([D + 1, 512], F32R, tag="ysb")
                    nc.vector.tensor_copy(ysb[:], yu[:])
                    r_tmp = small.tile([1, 512], F32R, tag="rtmp")
                    nc.vector.reciprocal(r_tmp[:], ysb[D : D + 1, :])
                    r_bc = ps_y.tile([128, 512], F32, tag="y")
                    nc.tensor.matmul(
                        r_bc[:], ones1_sb[:], r_tmp[:], start=True, stop=True
                    )
                    rows = slice(64 * hi, 64 * hi + 64)
                    nc.vector.tensor_mul(
                        y_acc[rows, dp, dqb_sl], ysb[0:D, :], r_bc[rows, :]
                    )

            def emit_attention_qb(qb):
                ktl = _ktiles_for_qblock(qb)
                for p in range(NCH):  # c-chunk = head pair (2p, 2p+1)
                    yus = [
                        ps_y.tile([D + 1, 512], F32, tag="y", name=f"yu_{qb}_{p}_{i}")
                        for i in range(2)
                    ]
                    for ki, (region, j) in enumerate(ktl):
                        qts = _active_qts(region, j, qb)
                        qlo, qw = qts[0], len(qts)
                        q_sl = slice(128 * qlo, 128 * (qlo + qw))
                        rel_sl = slice(
                            128 * (qlo - 4 * qb), 128 * (qlo - 4 * qb + qw)
                        )
                        nq = 128 * qw
                        kbase = (0 if region == "e1" else T) + 128 * j
                        st_glob = kbase // 128

                        if ki == 1 and pending:
                            _emit_division(*pending.pop(0))

                        sp = ps_s.tile([128, 2, 512], F32, tag="s")
                        for hi in range(2):
                            lo = 64 * hi
                            nc.tensor.matmul(
                                sp[:, hi, 0:nq],
                                ktsb[lo : lo + 64, p, kbase : kbase + 128],
                                qT_sb[lo : lo + 64, p, q_sl],
                                start=True,
                                stop=True,
                            )
                        pt = pt_pool.tile([128, 2, 512], BF16, tag="pt")
                        nc.scalar.activation(
                            pt[:, :, 0:nq],
                            sp[:, :, 0:nq],
                            mybir.ActivationFunctionType.Exp,
                        )
                        for qt in qts:
                            if (region, j, qt) in MASK_IDX:
                                mi = MASK_IDX[(region, j, qt)]
                                rel = slice(128 * (qt - qlo), 128 * (qt - qlo + 1))
                                mb = mask_sb[:, mi : mi + 1, :].broadcast_to(
                                    [128, 2, 128]
                                )
                                nc.vector.tensor_mul(pt[:, :, rel], pt[:, :, rel], mb)
                        for hi in range(2):
                            nc.tensor.matmul(
                                yus[hi][:, rel_sl],
                                vres[:, st_glob, 2 * p + hi, :],
                                pt[:, hi, 0:nq],
                                start=(ki == 0),
                                stop=(ki == len(ktl) - 1),
                                skip_group_check=True,
                            )
                    pending.append((qb, p, _division_stage1(yus)))

            # ===== phase A: projections (compensated fp8 DoubleRow) ============
            def dr_terms(w8_sb, wlo_sb, x8_t, xlo_t):
                return ((w8_sb, x8_t), (wlo_sb, x8_t), (w8_sb, xlo_t))

            def mid_group(out_ap, x8_t, xlo_t):
                """attn-lora mid: sum_c A[c, :]^T x[c, :] -> [RPAD, 512]."""
                i = 0
                for cp in range(NCP):
                    for lh, rh in dr_terms(la8_sb, lalo_sb, x8_t, xlo_t):
                        nc.tensor.matmul(
                            out_ap,
                            lh[:, cp, :, :],
                            rh[:, cp, :, :],
                            start=(i == 0),
                            stop=(i == 3 * NCP - 1),
                            perf_mode=DR,
                        )
                        i += 1

            with tc.tile_pool(name="wk", bufs=1) as wk_pool, tc.tile_pool(
                name="wv", bufs=1
            ) as wv_pool, tc.tile_pool(name="xa", bufs=2) as xa_pool:
                wk8_sb = wk_pool.tile([128, NCP, 2, C], F8E4)
                wklo_sb = wk_pool.tile([128, NCP, 2, C], F8E4)
                wv8_sb = wv_pool.tile([128, NCP, 2, C], F8E4)
                wvlo_sb = wv_pool.tile([128, NCP, 2, C], F8E4)
                nc.sync.dma_start(out=wk8_sb[:], in_=wk8[:])
                nc.sync.dma_start(out=wklo_sb[:], in_=wklo[:])
                nc.sync.dma_start(out=wv8_sb[:], in_=wv8[:])
                nc.sync.dma_start(out=wvlo_sb[:], in_=wvlo[:])

                # ---- Q^T projection (own T rows), resident ----
                with tc.tile_pool(name="wq", bufs=1) as wq_pool:
                    wq8_sb = wq_pool.tile([128, NCP, 2, C], F8E4)
                    wqlo_sb = wq_pool.tile([128, NCP, 2, C], F8E4)
                    nc.sync.dma_start(out=wq8_sb[:], in_=wq8[:])
                    nc.sync.dma_start(out=wqlo_sb[:], in_=wqlo[:])
                    for s in range(2):
                        sl = slice(s * 512, (s + 1) * 512)
                        xq8_t = xa_pool.tile([128, NCP, 2, 512], F8E4, tag="x8")
                        xqlo_t = xa_pool.tile([128, NCP, 2, 512], F8E4, tag="xlo")
                        nc.sync.dma_start(out=xq8_t[:], in_=xq8[:, :, :, sl])
                        nc.sync.dma_start(out=xqlo_t[:], in_=xqlo[:, :, :, sl])
                        tmq_ps = ps_s.tile([128, 2, 512], F32, tag="s")
                        mid_group(tmq_ps[0:RPAD, 0, :], xq8_t, xqlo_t)
                        tmq_sb = small.tile([RANK, 512], BF16, tag="tmq")
                        nc.vector.tensor_scalar_mul(
                            tmq_sb[:], tmq_ps[0:RANK, 0, :], 1.0 / WSC
                        )
                        for mp in range(4):
                            qps = ps_s.tile([128, 2, 512], F32, tag="s")
                            for h2 in range(2):
                                m = 2 * mp + h2
                                cols = slice(128 * m, 128 * (m + 1))
                                i = 0
                                for lh, rh in dr_terms(
                                    wq8_sb, wqlo_sb, xq8_t, xqlo_t
                                ):
                                    for cp in range(NCP):
                                        nc.tensor.matmul(
                                            qps[:, h2, :],
                                            lh[:, cp, :, cols],
                                            rh[:, cp, :, :],
                                            start=(i == 0),
                                            stop=False,
                                            perf_mode=DR,
                                        )
                                        i += 1
                                nc.tensor.matmul(
                                    qps[:, h2, :],
                                    lb_qk_sb[:, cols],
                                    tmq_sb[:],
                                    start=False,
                                    stop=True,
                                )
                            nc.scalar.mul(
                                qT_sb[:, 2 * mp : 2 * mp + 2, sl], qps[:], 1.0 / WSC
                            )

                # ---- K^T and V per seq block ----
                def emit_kv_block(s, stage_on_act):
                    sl = slice(s * 512, (s + 1) * 512)
                    x8_t = xa_pool.tile([128, NCP, 2, 512], F8E4, tag="x8")
                    xlo_t = xa_pool.tile([128, NCP, 2, 512], F8E4, tag="xlo")
                    nc.sync.dma_start(out=x8_t[:], in_=x8[:, :, :, sl])
                    nc.sync.dma_start(out=xlo_t[:], in_=xlo[:, :, :, sl])
                    tsl = None
                    if s >= 2:  # e2 rows: attn lora mid
                        tsl = slice((s - 2) * 512, (s - 1) * 512)
                        tmp_ps = ps_s.tile([128, 2, 512], F32, tag="s")
                        mid_group(tmp_ps[0:RPAD, 0, :], x8_t, xlo_t)
                        nc.vector.tensor_scalar_mul(
                            tmp_kv[:, tsl], tmp_ps[0:RANK, 0, :], 1.0 / WSC
                        )
                    for mp in range(4):  # kcol tile pairs
                        kps = ps_s.tile([128, 2, 512], F32, tag="s")
                        for h2 in range(2):
                            m = 2 * mp + h2
                            cols = slice(128 * m, 128 * (m + 1))
                            i = 0
                            for lh, rh in dr_terms(wk8_sb, wklo_sb, x8_t, xlo_t):
                                for cp in range(NCP):
                                    nc.tensor.matmul(
                                        kps[:, h2, :],
                                        lh[:, cp, :, cols],
                                        rh[:, cp, :, :],
                                        start=(i == 0),
                                        stop=(i == 3 * NCP - 1 and s < 2),
                                        perf_mode=DR,
                                    )
                                    i += 1
                            if s >= 2:
                                nc.tensor.matmul(
                                    kps[:, h2, :],
                                    lb_qk_sb[:, C + 128 * m : C + 128 * (m + 1)],
                                    tmp_kv[:, tsl],
                                    start=False,
                                    stop=True,
                                )
                        dst = ktsb[:, 2 * mp : 2 * mp + 2, sl]
                        if stage_on_act:
                            nc.scalar.mul(dst, kps[:], 1.0 / WSC)
                        else:
                            nc.vector.tensor_scalar_mul(dst, kps[:], 1.0 / WSC)
                    for st in range(4):  # V: 128-row seq tiles within block
                        ssl = slice(128 * st, 128 * (st + 1))
                        vps = ps_s.tile([128, 2, 512], F32, tag="s")
                        for vc in range(2):
                            vsl = slice(512 * vc, 512 * (vc + 1))
                            i = 0
                            for lh, rh in (
                                (x8_t, wv8_sb), (xlo_t, wv8_sb), (x8_t, wvlo_sb),
                            ):
                                for cp in range(NCP):
                                    nc.tensor.matmul(
                                        vps[:, vc, :],
                                        lh[:, cp, :, ssl],
                                        rh[:, cp, :, vsl],
                                        start=(i == 0),
                                        stop=(i == 3 * NCP - 1 and s < 2),
                                        perf_mode=DR,
                                    )
                                    i += 1
                            if s >= 2:
                                base = (s - 2) * 512 + 128 * st
                                nc.tensor.matmul(
                                    vps[:, vc, :],
                                    tmp_kv[:, base : base + 128],
                                    lb_v_sb[:, vsl],
                                    start=False,
                                    stop=True,
                                )
                        dst = vres[:, 4 * s + st, :, 0:D]
                        vsrc = vps[:].rearrange("p v (h d) -> p (v h) d", h=8)
                        if stage_on_act:
                            nc.scalar.mul(dst, vsrc, 1.0 / WSC)
                        else:
                            nc.vector.tensor_scalar_mul(dst, vsrc, 1.0 / WSC)

                emit_kv_block(0, True)
                emit_kv_block(1, True)
                emit_kv_block(2, True)
                emit_attention_qb(0)
                emit_kv_block(3, False)

            # ===== phase B: output projection + qb1 ===========================
            bpool = ctx.enter_context(tc.tile_pool(name="bpool", bufs=1))
            wproj_sb = bpool.tile([128, NCH, C], BF16)
            nc.sync.dma_start(out=wproj_sb[:], in_=wproj[:])
            ost_pool = ctx.enter_context(tc.tile_pool(name="ost", bufs=2))

            def emit_proj_qb(qb):
                while pending:
                    _emit_division(*pending.pop(0))
                qb_sl = slice(qb * 512, (qb + 1) * 512)
                tm2_ps = ps_s.tile([128, 2, 512], F32, tag="s")
                for ch in range(NCH):
                    nc.tensor.matmul(
                        tm2_ps[0:RANK, 0, :],
                        la_proj_sb[:, ch, :],
                        y_acc[:, ch, qb_sl],
                        start=(ch == 0),
                        stop=(ch == NCH - 1),
                    )
                tm2_sb = small.tile([RANK, 512], BF16, tag="tm2")
                nc.vector.tensor_copy(tm2_sb[:], tm2_ps[0:RANK, 0, :])
                for qs in range(4):
                    qrow = 512 * qb + 128 * qs
                    ops = ps_s.tile([128, 2, 512], F32, tag="s")
                    for co in range(2):
                        cos = slice(512 * co, 512 * (co + 1))
                        for ch in range(NCH):
                            nc.tensor.matmul(
                                ops[:, co, :],
                                y_acc[:, ch, qrow : qrow + 128],
                                wproj_sb[:, ch, cos],
                                start=(ch == 0),
                                stop=False,
                            )
                        nc.tensor.matmul(
                            ops[:, co, :],
                            tm2_sb[:, 128 * qs : 128 * (qs + 1)],
                            lb_proj_sb[:, cos],
                            start=False,
                            stop=True,
                        )
                    ost = ost_pool.tile([128, 2, 512], F32, tag="ost")
                    nc.vector.tensor_copy(ost[:], ops[:])
                    nc.sync.dma_start(
                        out=yout[qrow : qrow + 128, :],
                        in_=ost[:].rearrange("p a b -> p (a b)"),
                    )

            emit_proj_qb(0)
            emit_attention_qb(1)
            emit_proj_qb(1)
    return nc


_PROGRAM = None


def _get_program():
    global _PROGRAM
    if _PROGRAM is None:
        _PROGRAM = _build_program()
    return _PROGRAM


# ---------------------------------------------------------------------------
# Host side
# ---------------------------------------------------------------------------
def _delayed_mask_np(t):
    ones = np.ones((t, t), dtype=bool)
    m11 = np.tril(ones) & np.triu(ones, -(LOOKAHEAD + OVERLAP))
    m12 = np.tril(ones, -LOOKAHEAD)
    m21 = np.tril(ones, LOOKAHEAD) & np.triu(ones, -OVERLAP)
    m22 = np.tril(ones)
    return np.block([[m11, m12], [m21, m22]])


def _fp8_pair(a):
    hi = a.astype(FP8NP)
    lo = (a - hi.astype(np.float32)).astype(FP8NP)
    return hi, lo


def _cp_layout(m):
    """[C, N] -> [128, NCP, 2, N] with c = 256*cp + 128*i + p."""
    n = m.shape[1]
    return np.ascontiguousarray(m.reshape(NCP, 2, 128, n).transpose(2, 0, 1, 3))


def kernel(
    e1,
    e2,
    W_attn,
    W_proj,
    lora_A_attn,
    lora_B_attn,
    lora_A_proj,
    lora_B_proj,
    _trace=False,
):
    f32 = np.float32
    bf16 = ml_dtypes.bfloat16
    e1 = np.asarray(e1, f32)
    e2 = np.asarray(e2, f32)
    W_attn = np.asarray(W_attn, f32)
    W_proj = np.asarray(W_proj, f32)
    lora_A_attn = np.asarray(lora_A_attn, f32)
    lora_B_attn = np.asarray(lora_B_attn, f32)
    lora_A_proj = np.asarray(lora_A_proj, f32)
    lora_B_proj = np.asarray(lora_B_proj, f32)
    nc = _get_program()
    M = _delayed_mask_np(T)

    # --- role-independent prep (once) ---
    wq8, wqlo = _fp8_pair(_cp_layout(W_attn[:, :C] * (WSC * QSCALE)))
    wk8, wklo = _fp8_pair(_cp_layout(W_attn[:, C : 2 * C] * WSC))
    wv8, wvlo = _fp8_pair(_cp_layout(W_attn[:, 2 * C :] * WSC))
    la_pad = np.zeros((C, RPAD), f32)
    la_pad[:, :RANK] = lora_A_attn * WSC
    la8, lalo = _fp8_pair(_cp_layout(la_pad))
    la_proj = np.ascontiguousarray(
        lora_A_proj.reshape(NCH, 128, RANK).transpose(1, 0, 2)
    ).astype(bf16)
    wproj_r = np.ascontiguousarray(
        W_proj.reshape(NCH, 128, C).transpose(1, 0, 2)
    ).astype(bf16)
    lb_v = (np.ascontiguousarray(lora_B_attn[:, 2 * C :]) * (LSCALE * WSC)).astype(
        bf16
    )
    ones1 = np.ones((1, 128), f32)

    lbqk = {}
    lbp = {}
    for r in (0, 1):
        q = np.array(lora_B_attn[:, :C], dtype=f32) * (LSCALE * WSC * QSCALE)
        if r == 0:
            q[:] = 0.0
        k = lora_B_attn[:, C : 2 * C] * (LSCALE * WSC)
        lbqk[r] = np.concatenate([q, k], axis=1).astype(bf16)
        p = np.array(lora_B_proj, dtype=f32) * LSCALE
        if r == 0:
            p[:] = 0.0
        lbp[r] = p.astype(bf16)

    masks_r = {}
    for r in (0, 1):
        mk = np.empty((NMASK, 128, 128), dtype=bf16)
        for i, (region, j, qt) in enumerate(MASK_TILES):
            qg = r * T + 128 * qt
            kg = (0 if region == "e1" else T) + 128 * j
            mk[i] = M[qg : qg + 128, kg : kg + 128].T.astype(f32)
        masks_r[r] = mk

    in_maps = []
    x_cache = None
    for core in range(8):
        b, r = core // 2, core % 2
        if r == 0:
            x = np.concatenate([e1[b], e2[b]], axis=0)  # [2T, C]
            xT = np.ascontiguousarray(x.T)
            x_cache = _fp8_pair(_cp_layout(xT))
        x8b, xlob = x_cache
        qsl = slice(r * T, (r + 1) * T)
        in_maps.append({
            "x8": x8b,
            "xlo": xlob,
            "xq8": np.ascontiguousarray(x8b[:, :, :, qsl]),
            "xqlo": np.ascontiguousarray(xlob[:, :, :, qsl]),
            "wq8": wq8, "wqlo": wqlo,
            "wk8": wk8, "wklo": wklo,
            "wv8": wv8, "wvlo": wvlo,
            "la8": la8, "lalo": lalo,
            "lb_qk": lbqk[r],
            "lb_v": lb_v,
            "la_proj": la_proj,
            "lb_proj": lbp[r],
            "wproj": wproj_r,
            "masks": masks_r[r],
            "ones1": ones1,
        })

    res = run_bass_kernel_spmd(nc, in_maps, core_ids=list(range(8)), trace=_trace)
    y1 = np.stack([res.results[2 * b]["yout"] for b in range(B)])
    y2 = np.stack([res.results[2 * b + 1]["yout"] for b in range(B)])
    if _trace:
        kernel.last_results = res
    return y1, y2


# revision 49
# speedup vs baseline: 1.2286x; 1.0133x over previous
"""Trainium2 Bass kernel for nn_DelayedSelfAttention (B=4, T=1024, C=1024, H=16).

Sharding: 8 cores = 4 batches x 2 sequence-halves.  Core c handles batch
c//2 and query rows [r*T, (r+1)*T) of the concatenated [2T] sequence
(r = c%2).  Each core computes K/V for the full 2T sequence (duplicated
kv-projection -- cheaper than any collective on this fabric), attention
for its T query rows over all 16 heads, and the output projection for
its rows.  Role asymmetry (mask values, q/proj LoRA) is pushed into
per-core input data so a single SPMD program serves all cores.

v2 vs the spill-to-DRAM baseline:
 - K^T and V stay RESIDENT in SBUF (no DRAM spill + reload).
 - QKV projections run as compensated fp8e4m3 DoubleRow matmuls:
   x ~ x8 + xlo, W ~ W8 + Wlo (host-quantized; weights prescaled by 64
   to clear the e4m3 subnormal range, staging copies scale by 1/64).
   Three DR terms (x8W8 + xloW8 + x8Wlo) cover a 256-deep contraction
   in 1.5 row-passes vs bf16's 2 -- ~25% tensor-engine saving at ~0.25%
   error (compensation cancels first-order quantization).
 - exp batched per head-pair ([128, 2, nq] PSUM duos), masks multiplied
   with a stride-0 head-broadcast, head-phase staging copies on the
   (otherwise idle) scalar engine.
 - emission order software-pipelines the phases: Q-proj, K/V blocks
   s0..s2, q-block-0 attention overlapping the s3 projection, then the
   qb0 output projection, qb1 attention, qb1 projection.
"""

import contextlib
import sys

for _p in ("/opt/trn_rl_repo", "/root/.axon_site/_ro/trn_rl_repo"):
    if _p not in sys.path:
        sys.path.insert(0, _p)

import ml_dtypes
import numpy as np

import concourse.bass as bass
import concourse.mybir as mybir
import concourse.tile as tile_mod
from concourse.bass_utils import run_bass_kernel_spmd
from concourse.tile import TileContext
from concourse.vector_clock import ScopedClock

# ---------------------------------------------------------------------------
# Workaround: this walrus build supports a single semaphore wait per
# instruction.  Split multi-wait instructions into same-engine NoOps each
# carrying one wait (identical sequencer semantics).
# ---------------------------------------------------------------------------
_ws_counter = [0]


def _fresh_name():
    _ws_counter[0] += 1
    return f"I-waitsplit-{_ws_counter[0]}"


def _split_inst_waits(inst):
    si = inst.sync_info
    if si is None:
        return []
    waits = list(si.on_wait or [])
    if len(waits) <= 1:
        return []
    nops = []
    for w in waits[:-1]:
        nop = mybir.InstNoOp(name=_fresh_name())
        nop.engine = inst.engine
        nop.sync_info = mybir.SyncInfo(on_wait=[w], on_update=[])
        nops.append(nop)
    inst.sync_info = mybir.SyncInfo(
        on_wait=[waits[-1]], on_update=list(si.on_update or [])
    )
    return nops


_orig_lower = tile_mod.TileContext._lower_ordered_insts


def _patched_lower(self, ordered):
    for bb_name in list(ordered.keys()):
        new = []
        for inst in ordered[bb_name]:
            new.extend(_split_inst_waits(inst))
            new.append(inst)
        ordered[bb_name] = new
    return _orig_lower(self, ordered)


def _patched_drain_and_barrier(self, tick_clock, wait_clock):
    nc = self.nc
    drain_inst = nc.sync.drain()
    wait_clock.add_sem_waits(
        drain_inst.ins, ScopedClock({None: tick_clock.global_clock})
    )
    nops = _split_inst_waits(drain_inst.ins)
    if nops:
        first_wait = drain_inst.ins.sync_info
        drain_inst.ins.sync_info = mybir.SyncInfo(on_wait=[], on_update=[])
        for nop in nops:
            n2 = nc.sync.nop(nofuse=True)
            n2.ins.sync_info = nop.sync_info
        d2 = nc.sync.drain()
        d2.ins.sync_info = first_wait

    nc.all_engine_barrier()
    assert self.sems is not None
    popped = nc._tile_sem_poison_stack.pop()
    assert popped is self._sem_poison
    nc.clear_and_free_semaphores(list(self.sems.allocated().values()))
    nc.all_engine_barrier()


def _apply_tile_patch():
    if tile_mod.TileContext._lower_ordered_insts is not _patched_lower:
        tile_mod.TileContext._lower_ordered_insts = _patched_lower
        tile_mod.TileContext._drain_and_barrier = _patched_drain_and_barrier


# ---------------------------------------------------------------------------
# Problem constants (hardcoded per the task contract).
# ---------------------------------------------------------------------------
B, T, C, H = 4, 1024, 1024, 16
D = C // H  # 64
SEQ = 2 * T
LOOKAHEAD, OVERLAP = 64, 64
RANK, ALPHA = 8, 16.0
RPAD = 16  # lora-A stationary padded (dual-fp8 ldweights needs width >= 16)
LSCALE = ALPHA / RANK  # 2.0
QSCALE = 1.0 / np.sqrt(D)  # 1/8
WSC = 64.0  # fp8 weight prescale (cleared by 1/WSC at staging)
NCH = C // 128  # 8 c-chunks
NCP = NCH // 2  # 4 c-chunk-pairs (DoubleRow)
NQT = T // 128  # 8 q-subtiles per core
F32 = mybir.dt.float32
F32R = mybir.dt.float32r
BF16 = mybir.dt.bfloat16
F8E4 = mybir.dt.float8e4
FP8NP = ml_dtypes.float8_e4m3fn
DR = mybir.MatmulPerfMode.DoubleRow


# Trace-time tiling structure, shared by host (mask packing) and device.
def _ktiles_for_qblock(qb):
    """k-tiles (region, j) touched by q-subtiles [4qb, 4qb+4)."""
    qts = range(4 * qb, 4 * qb + 4)
    e1 = sorted({j for qt in qts for j in (qt - 1, qt, qt + 1) if 0 <= j < NQT})
    e2 = sorted({j for qt in qts for j in range(qt + 1)})
    return [("e1", j) for j in e1] + [("e2", j) for j in e2]


def _active_qts(region, j, qb):
    if region == "e1":
        qts = [qt for qt in range(4 * qb, 4 * qb + 4) if j in (qt - 1, qt, qt + 1)]
    else:
        qts = [qt for qt in range(4 * qb, 4 * qb + 4) if j <= qt]
    assert qts == list(range(qts[0], qts[-1] + 1))
    return qts


def _mask_tiles():
    out = []
    for qt in range(NQT):
        for j in (qt - 1, qt, qt + 1):
            if 0 <= j < NQT:
                out.append(("e1", j, qt))
        for j in (qt - 1, qt):
            if j >= 0:
                out.append(("e2", j, qt))
    return out


MASK_TILES = _mask_tiles()  # 37 tiles
MASK_IDX = {k: i for i, k in enumerate(MASK_TILES)}
NMASK = len(MASK_TILES)


# ---------------------------------------------------------------------------
# Device program
# ---------------------------------------------------------------------------
def _build_program():
    _apply_tile_patch()
    nc = bass.Bass("TRN2", target_bir_lowering=False, debug=False, num_devices=8)

    def din(name, shape, dt=F32R):
        return nc.dram_tensor(name, list(shape), dt, kind="ExternalInput").ap()

    x8 = din("x8", (128, NCP, 2, SEQ), dt=F8E4)
    xlo = din("xlo", (128, NCP, 2, SEQ), dt=F8E4)
    xq8 = din("xq8", (128, NCP, 2, T), dt=F8E4)
    xqlo = din("xqlo", (128, NCP, 2, T), dt=F8E4)
    wq8 = din("wq8", (128, NCP, 2, C), dt=F8E4)
    wqlo = din("wqlo", (128, NCP, 2, C), dt=F8E4)
    wk8 = din("wk8", (128, NCP, 2, C), dt=F8E4)
    wklo = din("wklo", (128, NCP, 2, C), dt=F8E4)
    wv8 = din("wv8", (128, NCP, 2, C), dt=F8E4)
    wvlo = din("wvlo", (128, NCP, 2, C), dt=F8E4)
    la8 = din("la8", (128, NCP, 2, RPAD), dt=F8E4)
    lalo = din("lalo", (128, NCP, 2, RPAD), dt=F8E4)
    lb_qk = din("lb_qk", (RANK, 2 * C), dt=BF16)  # scaled, role-zeroed q
    lb_v = din("lb_v", (RANK, C), dt=BF16)
    la_proj = din("la_proj", (128, NCH, RANK), dt=BF16)
    lb_proj = din("lb_proj", (RANK, C), dt=BF16)  # zeroed for role 0
    wproj = din("wproj", (128, NCH, C), dt=BF16)
    masks = din("masks", (NMASK, 128, 128), dt=BF16)
    ones1 = din("ones1", (1, 128))
    yout = nc.dram_tensor("yout", [T, C], F32, kind="ExternalOutput").ap()

    with TileContext(nc) as tc:
        ctx = contextlib.ExitStack()
        with ctx:
            ctx.enter_context(
                nc.allow_low_precision(reason="float32r is full-width fp32 storage")
            )
            # --- persistent SBUF ---
            persist = ctx.enter_context(tc.tile_pool(name="persist", bufs=1))
            ktsb = persist.tile([128, NCH, SEQ], BF16)      # resident K^T
            vres = persist.tile([128, 16, H, D + 1], BF16)  # resident V + ones col
            qT_sb = persist.tile([128, NCH, T], BF16)       # resident Q^T (prescaled)
            y_acc = persist.tile([128, NCH, T], BF16)       # normalized y
            mask_sb = persist.tile([128, NMASK, 128], BF16)
            tmp_kv = persist.tile([RANK, T], BF16)          # e2 attn-lora mid
            la8_sb = persist.tile([128, NCP, 2, RPAD], F8E4)
            lalo_sb = persist.tile([128, NCP, 2, RPAD], F8E4)
            lb_qk_sb = persist.tile([RANK, 2 * C], BF16)
            lb_v_sb = persist.tile([RANK, C], BF16)
            la_proj_sb = persist.tile([128, NCH, RANK], BF16)
            lb_proj_sb = persist.tile([RANK, C], BF16)
            ones1_sb = persist.tile([1, 128], F32R)

            nc.vector.memset(vres[:, :, :, D : D + 1], 1.0)  # ones column

            # --- PSUM pools: 4 (yu / r_bc) + 2x2 (score duos) = 8 banks
            ps_y = ctx.enter_context(tc.tile_pool(name="ps_y", bufs=4, space="PSUM"))
            ps_s = ctx.enter_context(tc.tile_pool(name="ps_s", bufs=2, space="PSUM"))

            small = ctx.enter_context(tc.tile_pool(name="small", bufs=1))
            pt_pool = ctx.enter_context(tc.tile_pool(name="pt", bufs=10))
            ysb_pool = ctx.enter_context(tc.tile_pool(name="ysb", bufs=3))

            pending = []
            holders = {}

            def _division_stage1(dyus):
                ysbs = []
                for hi in range(2):
                    ysb = ysb_pool.tile([D + 1, 512], F32R, tag="ysb")
                    nc.vector.tensor_copy(ysb[:], dyus[hi][:])
                    r_tmp = ysb_pool.tile([1, 512], F32R, tag="rt")
                    nc.vector.reciprocal(r_tmp[:], ysb[D : D + 1, :])
                    ysbs.append((ysb, r_tmp))
                return ysbs

            def _division_stage2(dqb, dp, ysbs):
                dqb_sl = slice(dqb * 512, (dqb + 1) * 512)
                y_acc = holders["y_acc"]
                for hi in range(2):
                    ysb, r_tmp = ysbs[hi]
                    r_bc = ps_y.tile([128, 512], F32, tag="y")
                    nc.tensor.matmul(
                        r_bc[:], ones1_sb[:], r_tmp[:],
                        start=True, stop=True,
                    )
                    rows = slice(64 * hi, 64 * hi + 64)
                    nc.vector.tensor_mul(
                        y_acc[rows, dp, dqb_sl], ysb[0:D, :], r_bc[rows, :]
                    )

            def emit_attention_p(qb, p, bracket=()):
                """Attention for q-block qb, c-chunk p (head pair 2p, 2p+1).

                AV matmuls lag scores by one k-tile so the tensor engine has
                work while exp/mask chains drain; bracket thunks (projection
                work) are popped one per k-tile to fill remaining gaps.
                """
                items = list(bracket)
                ktl = _ktiles_for_qblock(qb)
                yus = [
                    ps_y.tile([D + 1, 512], F32, tag="y", name=f"yu_{qb}_{p}_{i}")
                    for i in range(2)
                ]
                avq = []

                def flush_av():
                    ki0, pt0, nq0, rel0, st0 = avq.pop(0)
                    for hi in range(2):
                        nc.tensor.matmul(
                            yus[hi][:, rel0],
                            vres[:, st0, 2 * p + hi, :],
                            pt0[:, hi, 0:nq0],
                            start=(ki0 == 0),
                            stop=(ki0 == len(ktl) - 1),
                            skip_group_check=True,
                        )

                for ki, (region, j) in enumerate(ktl):
                    qts = _active_qts(region, j, qb)
                    qlo, qw = qts[0], len(qts)
                    q_sl = slice(128 * qlo, 128 * (qlo + qw))
                    rel_sl = slice(128 * (qlo - 4 * qb), 128 * (qlo - 4 * qb + qw))
                    nq = 128 * qw
                    kbase = (0 if region == "e1" else T) + 128 * j
                    st_glob = kbase // 128

                    if ki == 1 and pending:
                        _division_stage2(*pending.pop(0))
                    if items and ki >= 1:
                        items.pop(0)()

                    sp = ps_s.tile([128, 2, 512], F32, tag="s")
                    for hi in range(2):
                        lo = 64 * hi
                        nc.tensor.matmul(
                            sp[:, hi, 0:nq],
                            ktsb[lo : lo + 64, p, kbase : kbase + 128],
                            qT_sb[lo : lo + 64, p, q_sl],
                            start=True,
                            stop=True,
                        )
                    pt = pt_pool.tile([128, 2, 512], BF16, tag="pt")
                    nc.scalar.activation(
                        pt[:, :, 0:nq],
                        sp[:, :, 0:nq],
                        mybir.ActivationFunctionType.Exp,
                    )
                    for qt in qts:
                        if (region, j, qt) in MASK_IDX:
                            mi = MASK_IDX[(region, j, qt)]
                            rel = slice(128 * (qt - qlo), 128 * (qt - qlo + 1))
                            mb = mask_sb[:, mi : mi + 1, :].broadcast_to(
                                [128, 2, 128]
                            )
                            nc.vector.tensor_mul(pt[:, :, rel], pt[:, :, rel], mb)
                    avq.append((ki, pt, nq, rel_sl, st_glob))
                    flush_av()
                while avq:
                    flush_av()
                for it in items:
                    it()
                pending.append((qb, p, _division_stage1(yus)))

            # ===== phase A: projections (compensated fp8 DoubleRow) ============
            def mid_group(out_ap, x8_t, xlo_t, sl):
                """attn-lora mid: sum_c A[c, :]^T x[c, sl] -> [RPAD, 512]."""
                i = 0
                for cp in range(NCP):
                    for lh, rh in (
                        (la8_sb, x8_t), (lalo_sb, x8_t), (la8_sb, xlo_t),
                    ):
                        nc.tensor.matmul(
                            out_ap,
                            lh[:, cp, :, :],
                            rh[:, cp, :, sl],
                            start=(i == 0),
                            stop=(i == 3 * NCP - 1),
                            perf_mode=DR,
                        )
                        i += 1

            actx = contextlib.ExitStack()
            with actx:
                wk_pool = actx.enter_context(tc.tile_pool(name="wk", bufs=1))
                xa_pool = actx.enter_context(tc.tile_pool(name="xa", bufs=2))

                def load_xq(s):
                    sl = slice(s * 512, (s + 1) * 512)
                    xq8_t = xa_pool.tile([128, NCP, 2, 512], F8E4, tag="x8",
                                         name=f"xq8_{s}")
                    xqlo_t = xa_pool.tile([128, NCP, 2, 512], F8E4, tag="xlo",
                                          name=f"xqlo_{s}")
                    nc.sync.dma_start(out=xq8_t[:], in_=xq8[:, :, :, sl])
                    nc.sync.dma_start(out=xqlo_t[:], in_=xqlo[:, :, :, sl])
                    return xq8_t, xqlo_t

                # ---- Q^T projection first (own T rows), resident ----
                with tc.tile_pool(name="wq", bufs=1) as wq_pool:
                    wq8_sb = wq_pool.tile([128, NCP, 2, C], F8E4)
                    wqlo_sb = wq_pool.tile([128, NCP, 2, C], F8E4)
                    nc.sync.dma_start(out=wq8_sb[:], in_=wq8[:])
                    xqt = {s_: load_xq(s_) for s_ in range(2)}
                    nc.sync.dma_start(out=la8_sb[:], in_=la8[:])
                    nc.sync.dma_start(out=lalo_sb[:], in_=lalo[:])
                    nc.sync.dma_start(out=wqlo_sb[:], in_=wqlo[:])
                    nc.sync.dma_start(out=lb_qk_sb[:], in_=lb_qk[:])
                    # prefetch K weights while Q computes
                    wk8_sb = wk_pool.tile([128, NCP, 2, C], F8E4)
                    wklo_sb = wk_pool.tile([128, NCP, 2, C], F8E4)
                    nc.sync.dma_start(out=wk8_sb[:], in_=wk8[:])
                    nc.sync.dma_start(out=wklo_sb[:], in_=wklo[:])
                    nc.sync.dma_start(
                        out=mask_sb[:], in_=masks.rearrange("t p q -> p t q")
                    )
                    nc.sync.dma_start(out=lb_v_sb[:], in_=lb_v[:])
                    nc.sync.dma_start(out=la_proj_sb[:], in_=la_proj[:])
                    nc.sync.dma_start(out=lb_proj_sb[:], in_=lb_proj[:])
                    nc.sync.dma_start(out=ones1_sb[:], in_=ones1[:])

                    for s_ in range(2):
                        sl = slice(s_ * 512, (s_ + 1) * 512)
                        xq8_t, xqlo_t = xqt[s_]
                        tmq_ps = ps_s.tile([128, 2, 512], F32, tag="s")
                        mid_group(tmq_ps[0:RPAD, 0, :], xq8_t, xqlo_t,
                                  slice(0, 512))
                        tmq_sb = small.tile([RANK, 512], BF16, tag="tmq")
                        nc.vector.tensor_scalar_mul(
                            tmq_sb[:], tmq_ps[0:RANK, 0, :], 1.0 / WSC
                        )
                        for mp in range(4):
                            qps = ps_s.tile([128, 2, 512], F32, tag="s")
                            for h2 in range(2):
                                m = 2 * mp + h2
                                cols = slice(128 * m, 128 * (m + 1))
                                i = 0
                                for lh, rh in (
                                    (wq8_sb, xq8_t),
                                    (wqlo_sb, xq8_t),
                                    (wq8_sb, xqlo_t),
                                ):
                                    for cp in range(NCP):
                                        nc.tensor.matmul(
                                            qps[:, h2, :],
                                            lh[:, cp, :, cols],
                                            rh[:, cp, :, :],
                                            start=(i == 0),
                                            stop=False,
                                            perf_mode=DR,
                                        )
                                        i += 1
                                nc.tensor.matmul(
                                    qps[:, h2, :],
                                    lb_qk_sb[:, cols],
                                    tmq_sb[:],
                                    start=False,
                                    stop=True,
                                )
                            nc.scalar.mul(
                                qT_sb[:, 2 * mp : 2 * mp + 2, sl], qps[:], 1.0 / WSC
                            )

                wv_pool = actx.enter_context(tc.tile_pool(name="wv", bufs=1))
                wv8_sb = wv_pool.tile([128, NCP, 2, C], F8E4)
                wvlo_sb = wv_pool.tile([128, NCP, 2, C], F8E4)
                nc.sync.dma_start(out=wv8_sb[:], in_=wv8[:])
                nc.sync.dma_start(out=wvlo_sb[:], in_=wvlo[:])
                holders["y_acc"] = y_acc

                def load_x(s):
                    sl = slice(s * 512, (s + 1) * 512)
                    x8_t = xa_pool.tile([128, NCP, 2, 512], F8E4, tag="x8",
                                        name=f"x8_{s}")
                    xlo_t = xa_pool.tile([128, NCP, 2, 512], F8E4, tag="xlo",
                                         name=f"xlo_{s}")
                    nc.sync.dma_start(out=x8_t[:], in_=x8[:, :, :, sl])
                    nc.sync.dma_start(out=xlo_t[:], in_=xlo[:, :, :, sl])
                    return x8_t, xlo_t

                def emit_mid(s, x8_t, xlo_t):
                    tsl = slice((s - 2) * 512, (s - 1) * 512)
                    tmp_ps = ps_s.tile([128, 2, 512], F32, tag="s")
                    mid_group(tmp_ps[0:RPAD, 0, :], x8_t, xlo_t, slice(0, 512))
                    nc.vector.tensor_scalar_mul(
                        tmp_kv[:, tsl], tmp_ps[0:RANK, 0, :], 1.0 / WSC
                    )

                def emit_k_duo(s, mp, x8_t, xlo_t, stage_on_act=True):
                    sl = slice(s * 512, (s + 1) * 512)
                    tsl = slice((s - 2) * 512, (s - 1) * 512) if s >= 2 else None
                    kps = ps_s.tile([128, 2, 512], F32, tag="s")
                    for h2 in range(2):
                        m = 2 * mp + h2
                        cols = slice(128 * m, 128 * (m + 1))
                        i = 0
                        for lh, rh in (
                            (wk8_sb, x8_t), (wklo_sb, x8_t), (wk8_sb, xlo_t),
                        ):
                            for cp in range(NCP):
                                nc.tensor.matmul(
                                    kps[:, h2, :],
                                    lh[:, cp, :, cols],
                                    rh[:, cp, :, :],
                                    start=(i == 0),
                                    stop=(i == 3 * NCP - 1 and s < 2),
                                    perf_mode=DR,
                                )
                                i += 1
                        if s >= 2:
                            nc.tensor.matmul(
                                kps[:, h2, :],
                                lb_qk_sb[:, C + 128 * m : C + 128 * (m + 1)],
                                tmp_kv[:, tsl],
                                start=False,
                                stop=True,
                            )
                    dst = ktsb[:, 2 * mp : 2 * mp + 2, sl]
                    if stage_on_act:
                        nc.scalar.mul(dst, kps[:], 1.0 / WSC)
                    else:
                        nc.vector.tensor_scalar_mul(dst, kps[:], 1.0 / WSC)

                def emit_v_duo(s, st, x8_t, xlo_t, stage_on_act=True):
                    ssl = slice(128 * st, 128 * (st + 1))
                    vps = ps_s.tile([128, 2, 512], F32, tag="s")
                    for vc in range(2):
                        vsl = slice(512 * vc, 512 * (vc + 1))
                        i = 0
                        for lh, rh in (
                            (x8_t, wv8_sb), (xlo_t, wv8_sb), (x8_t, wvlo_sb),
                        ):
                            for cp in range(NCP):
                                nc.tensor.matmul(
                                    vps[:, vc, :],
                                    lh[:, cp, :, ssl],
                                    rh[:, cp, :, vsl],
                                    start=(i == 0),
                                    stop=(i == 3 * NCP - 1 and s < 2),
                                    perf_mode=DR,
                                )
                                i += 1
                        if s >= 2:
                            base = (s - 2) * 512 + 128 * st
                            nc.tensor.matmul(
                                vps[:, vc, :],
                                tmp_kv[:, base : base + 128],
                                lb_v_sb[:, vsl],
                                start=False,
                                stop=True,
                            )
                    dst = vres[:, 4 * s + st, :, 0:D]
                    vsrc = vps[:].rearrange("p v (h d) -> p (v h) d", h=8)
                    if stage_on_act:
                        nc.scalar.mul(dst, vsrc, 1.0 / WSC)
                    else:
                        nc.vector.tensor_scalar_mul(dst, vsrc, 1.0 / WSC)

                def emit_v_single(s, st, vc, x8_t, xlo_t, stage_on_act=True):
                    ssl = slice(128 * st, 128 * (st + 1))
                    vsl = slice(512 * vc, 512 * (vc + 1))
                    vps = ps_s.tile([128, 2, 512], F32, tag="s")
                    i = 0
                    for lh, rh in (
                        (x8_t, wv8_sb), (xlo_t, wv8_sb), (x8_t, wvlo_sb),
                    ):
                        for cp in range(NCP):
                            nc.tensor.matmul(
                                vps[:, 0, :],
                                lh[:, cp, :, ssl],
                                rh[:, cp, :, vsl],
                                start=(i == 0),
                                stop=(i == 3 * NCP - 1 and s < 2),
                                perf_mode=DR,
                            )
                            i += 1
                    if s >= 2:
                        base = (s - 2) * 512 + 128 * st
                        nc.tensor.matmul(
                            vps[:, 0, :],
                            tmp_kv[:, base : base + 128],
                            lb_v_sb[:, vsl],
                            start=False,
                            stop=True,
                        )
                    dst = vres[:, 4 * s + st, 8 * vc : 8 * vc + 8, 0:D]
                    vsrc = vps[:, 0, :].rearrange("p (h d) -> p h d", h=8)
                    if stage_on_act:
                        nc.scalar.mul(dst, vsrc, 1.0 / WSC)
                    else:
                        nc.vector.tensor_scalar_mul(dst, vsrc, 1.0 / WSC)

                # ---- blocks s0..s2 up front; s3 is emitted inside qb0
                # attention windows to keep the tensor engine fed ----
                for s_ in range(3):
                    x8_t, xlo_t = load_x(s_)
                    if s_ >= 2:
                        emit_mid(s_, x8_t, xlo_t)
                    for mp in range(4):
                        emit_k_duo(s_, mp, x8_t, xlo_t)
                    for st in range(4):
                        emit_v_duo(s_, st, x8_t, xlo_t)

                s3x = {}

                def s3_item(kind, idx):
                    def run():
                        if "x" not in s3x:
                            s3x["x"] = load_x(3)
                            emit_mid(3, *s3x["x"])
                        x8_t, xlo_t = s3x["x"]
                        if kind == "k":
                            emit_k_duo(3, idx, x8_t, xlo_t, stage_on_act=False)
                        else:
                            emit_v_duo(3, idx, x8_t, xlo_t, stage_on_act=False)
                    return run

                brackets0 = {
                    0: [s3_item("k", 0)],
                    1: [s3_item("v", 0)],
                    2: [s3_item("k", 1)],
                    3: [s3_item("v", 1)],
                    4: [s3_item("k", 2)],
                    5: [s3_item("v", 2)],
                    6: [s3_item("k", 3)],
                    7: [s3_item("v", 3)],
                }
                for p in range(8):
                    emit_attention_p(0, p, bracket=brackets0.get(p, ()))

            # ===== phase B: output projection + qb1 ===========================
            bpool = ctx.enter_context(tc.tile_pool(name="bpool", bufs=1))
            wproj_sb = bpool.tile([128, NCH, C], BF16)
            nc.sync.dma_start(out=wproj_sb[:], in_=wproj[:])
            ost_pool = ctx.enter_context(tc.tile_pool(name="ost", bufs=2))

            proj_state = {}

            def proj_tm2(qb):
                qb_sl = slice(qb * 512, (qb + 1) * 512)
                tm2_ps = ps_s.tile([128, 2, 512], F32, tag="s")
                for ch in range(NCH):
                    nc.tensor.matmul(
                        tm2_ps[0:RANK, 0, :],
                        la_proj_sb[:, ch, :],
                        y_acc[:, ch, qb_sl],
                        start=(ch == 0),
                        stop=(ch == NCH - 1),
                    )
                tm2_sb = small.tile([RANK, 512], BF16, tag="tm2")
                nc.vector.tensor_copy(tm2_sb[:], tm2_ps[0:RANK, 0, :])
                proj_state[qb] = tm2_sb

            def proj_qs(qb, qs):
                tm2_sb = proj_state[qb]
                qrow = 512 * qb + 128 * qs
                ops = ps_s.tile([128, 2, 512], F32, tag="s")
                for co in range(2):
                    cos = slice(512 * co, 512 * (co + 1))
                    for ch in range(NCH):
                        nc.tensor.matmul(
                            ops[:, co, :],
                            y_acc[:, ch, qrow : qrow + 128],
                            wproj_sb[:, ch, cos],
                            start=(ch == 0),
                            stop=False,
                        )
                    nc.tensor.matmul(
                        ops[:, co, :],
                        tm2_sb[:, 128 * qs : 128 * (qs + 1)],
                        lb_proj_sb[:, cos],
                        start=False,
                        stop=True,
                    )
                ost = ost_pool.tile([128, 2, 512], F32, tag="ost")
                nc.vector.tensor_copy(ost[:], ops[:])
                nc.sync.dma_start(
                    out=yout[qrow : qrow + 128, :],
                    in_=ost[:].rearrange("p a b -> p (a b)"),
                )

            proj0 = [lambda: proj_tm2(0)] + [
                (lambda qs=qs: proj_qs(0, qs)) for qs in range(4)
            ]
            brackets1 = {1: proj0[0:2], 2: proj0[2:3], 3: proj0[3:4], 4: proj0[4:5]}
            for p in range(8):
                emit_attention_p(1, p, bracket=brackets1.get(p, ()))

            while pending:
                _division_stage2(*pending.pop(0))
            proj_tm2(1)
            for qs in range(4):
                proj_qs(1, qs)
    return nc.dram_tensor(name, list(shape), dt, kind="ExternalInput").ap()

    x8 = din("x8", (128, NCP, 2, SEQ), dt=F8E4)
    xlo = din("xlo", (128, NCP, 2, SEQ), dt=F8E4)
    xq8 = din("xq8", (128, NCP, 2, T), dt=F8E4)
    xqlo = din("xqlo", (128, NCP, 2, T), dt=F8E4)
    wq8 = din("wq8", (128, NCP, 2, C), dt=F8E4)
    wqlo = din("wqlo", (128, NCP, 2, C), dt=F8E4)
    wk8 = din("wk8", (128, NCP, 2, C), dt=F8E4)
    wklo = din("wklo", (128, NCP, 2, C), dt=F8E4)
    wv8 = din("wv8", (128, NCP, 2, C), dt=F8E4)
    wvlo = din("wvlo", (128, NCP, 2, C), dt=F8E4)
    la8 = din("la8", (128, NCP, 2, RPAD), dt=F8E4)
    lalo = din("lalo", (128, NCP, 2, RPAD), dt=F8E4)
    lb_qk = din("lb_qk", (RANK, 2 * C), dt=BF16)  # scaled, role-zeroed q
    lb_v = din("lb_v", (RANK, C), dt=BF16)
    la_proj = din("la_proj", (128, NCH, RANK), dt=BF16)
    lb_proj = din("lb_proj", (RANK, C), dt=BF16)  # zeroed for role 0
    wproj = din("wproj", (128, NCH, C), dt=BF16)
    masks = din("masks", (NMASK, 128, 128), dt=BF16)
    ones1 = din("ones1", (1, 128))
    yout = nc.dram_tensor("yout", [T, C], F32, kind="ExternalOutput").ap()

    with TileContext(nc) as tc:
        ctx = contextlib.ExitStack()
        with ctx:
            ctx.enter_context(
                nc.allow_low_precision(reason="float32r is full-width fp32 storage")
            )
            # --- persistent SBUF ---
            persist = ctx.enter_context(tc.tile_pool(name="persist", bufs=1))
            ktsb = persist.tile([128, NCH, SEQ], BF16)      # resident K^T
            vres = persist.tile([128, 16, H, D + 1], BF16)  # resident V + ones col
            qT_sb = persist.tile([128, NCH, T], BF16)       # resident Q^T (prescaled)
            y_acc = persist.tile([128, NCH, T], BF16)       # normalized y
            mask_sb = persist.tile([128, NMASK, 128], BF16)
            tmp_kv = persist.tile([RANK, T], BF16)          # e2 attn-lora mid
            la8_sb = persist.tile([128, NCP, 2, RPAD], F8E4)
            lalo_sb = persist.tile([128, NCP, 2, RPAD], F8E4)
            lb_qk_sb = persist.tile([RANK, 2 * C], BF16)
            lb_v_sb = persist.tile([RANK, C], BF16)
            la_proj_sb = persist.tile([128, NCH, RANK], BF16)
            lb_proj_sb = persist.tile([RANK, C], BF16)
            ones1_sb = persist.tile([1, 128], F32R)

            nc.vector.memset(vres[:, :, :, D : D + 1], 1.0)  # ones column

            # --- PSUM pools: 4 (yu / r_bc) + 2x2 (score duos) = 8 banks
            ps_y = ctx.enter_context(tc.tile_pool(name="ps_y", bufs=4, space="PSUM"))
            ps_s = ctx.enter_context(tc.tile_pool(name="ps_s", bufs=2, space="PSUM"))

            small = ctx.enter_context(tc.tile_pool(name="small", bufs=1))
            pt_pool = ctx.enter_context(tc.tile_pool(name="pt", bufs=10))
            ysb_pool = ctx.enter_context(tc.tile_pool(name="ysb", bufs=3))

            # ===== attention helpers (phase-A-pool-free) =======================
            pending = []

            def _emit_division(dqb, dp, dyus):
                dqb_sl = slice(dqb * 512, (dqb + 1) * 512)
                for hi in range(2):
                    yu = dyus[hi]
                    ysb = ysb_pool.tile([D + 1, 512], F32R, tag="ysb")
                    nc.vector.tensor_copy(ysb[:], yu[:])
                    r_tmp = small.tile([1, 512], F32R, tag="rtmp")
                    nc.vector.reciprocal(r_tmp[:], ysb[D : D + 1, :])
                    r_bc = ps_y.tile([128, 512], F32, tag="y")
                    nc.tensor.matmul(
                        r_bc[:], ones1_sb[:], r_tmp[:], start=True, stop=True
                    )
                    rows = slice(64 * hi, 64 * hi + 64)
                    nc.vector.tensor_mul(
                        y_acc[rows, dp, dqb_sl], ysb[0:D, :], r_bc[rows, :]
                    )

            def emit_attention_qb(qb):
                ktl = _ktiles_for_qblock(qb)
                for p in range(NCH):  # c-chunk = head pair (2p, 2p+1)
                    yus = [
                        ps_y.tile([D + 1, 512], F32, tag="y", name=f"yu_{qb}_{p}_{i}")
                        for i in range(2)
                    ]
                    for ki, (region, j) in enumerate(ktl):
                        qts = _active_qts(region, j, qb)
                        qlo, qw = qts[0], len(qts)
                        q_sl = slice(128 * qlo, 128 * (qlo + qw))
                        rel_sl = slice(
                            128 * (qlo - 4 * qb), 128 * (qlo - 4 * qb + qw)
                        )
                        nq = 128 * qw
                        kbase = (0 if region == "e1" else T) + 128 * j
                        st_glob = kbase // 128

                        if ki == 1 and pending:
                            _emit_division(*pending.pop(0))

                        sp = ps_s.tile([128, 2, 512], F32, tag="s")
                        for hi in range(2):
                            lo = 64 * hi
                            nc.tensor.matmul(
                                sp[:, hi, 0:nq],
                                ktsb[lo : lo + 64, p, kbase : kbase + 128],
                                qT_sb[lo : lo + 64, p, q_sl],
                                start=True,
                                stop=True,
                            )
                        pt = pt_pool.tile([128, 2, 512], BF16, tag="pt")
                        nc.scalar.activation(
                            pt[:, :, 0:nq],
                            sp[:, :, 0:nq],
                            mybir.ActivationFunctionType.Exp,
                        )
                        for qt in qts:
                            if (region, j, qt) in MASK_IDX:
                                mi = MASK_IDX[(region, j, qt)]
                                rel = slice(128 * (qt - qlo), 128 * (qt - qlo + 1))
                                mb = mask_sb[:, mi : mi + 1, :].broadcast_to(
                                    [128, 2, 128]
                                )
                                nc.vector.tensor_mul(pt[:, :, rel], pt[:, :, rel], mb)
                        for hi in range(2):
                            nc.tensor.matmul(
                                yus[hi][:, rel_sl],
                                vres[:, st_glob, 2 * p + hi, :],
                                pt[:, hi, 0:nq],
                                start=(ki == 0),
                                stop=(ki == len(ktl) - 1),
                                skip_group_check=True,
                            )
                    pending.append((qb, p, _division_stage1(yus)))

            # ===== phase A: projections (compensated fp8 DoubleRow) ============
            def dr_terms(w8_sb, wlo_sb, x8_t, xlo_t):
                return ((w8_sb, x8_t), (wlo_sb, x8_t), (w8_sb, xlo_t))

            def mid_group(out_ap, x8_t, xlo_t):
                """attn-lora mid: sum_c A[c, :]^T x[c, :] -> [RPAD, 512]."""
                i = 0
                for cp in range(NCP):
                    for lh, rh in dr_terms(la8_sb, lalo_sb, x8_t, xlo_t):
                        nc.tensor.matmul(
                            out_ap,
                            lh[:, cp, :, :],
                            rh[:, cp, :, :],
                            start=(i == 0),
                            stop=(i == 3 * NCP - 1),
                            perf_mode=DR,
                        )
                        i += 1

            with tc.tile_pool(name="wk", bufs=1) as wk_pool, tc.tile_pool(
                name="wv", bufs=1
            ) as wv_pool, tc.tile_pool(name="xa", bufs=2) as xa_pool:
                wk8_sb = wk_pool.tile([128, NCP, 2, C], F8E4)
                wklo_sb = wk_pool.tile([128, NCP, 2, C], F8E4)
                wv8_sb = wv_pool.tile([128, NCP, 2, C], F8E4)
                wvlo_sb = wv_pool.tile([128, NCP, 2, C], F8E4)
                nc.sync.dma_start(out=wk8_sb[:], in_=wk8[:])
                nc.sync.dma_start(out=wklo_sb[:], in_=wklo[:])
                nc.sync.dma_start(out=wv8_sb[:], in_=wv8[:])
                nc.sync.dma_start(out=wvlo_sb[:], in_=wvlo[:])

                # ---- Q^T projection (own T rows), resident ----
                with tc.tile_pool(name="wq", bufs=1) as wq_pool:
                    wq8_sb = wq_pool.tile([128, NCP, 2, C], F8E4)
                    wqlo_sb = wq_pool.tile([128, NCP, 2, C], F8E4)
                    nc.sync.dma_start(out=wq8_sb[:], in_=wq8[:])
                    nc.sync.dma_start(out=wqlo_sb[:], in_=wqlo[:])
                    for s in range(2):
                        sl = slice(s * 512, (s + 1) * 512)
                        xq8_t = xa_pool.tile([128, NCP, 2, 512], F8E4, tag="x8")
                        xqlo_t = xa_pool.tile([128, NCP, 2, 512], F8E4, tag="xlo")
                        nc.sync.dma_start(out=xq8_t[:], in_=xq8[:, :, :, sl])
                        nc.sync.dma_start(out=xqlo_t[:], in_=xqlo[:, :, :, sl])
                        tmq_ps = ps_s.tile([128, 2, 512], F32, tag="s")
                        mid_group(tmq_ps[0:RPAD, 0, :], xq8_t, xqlo_t)
                        tmq_sb = small.tile([RANK, 512], BF16, tag="tmq")
                        nc.vector.tensor_scalar_mul(
                            tmq_sb[:], tmq_ps[0:RANK, 0, :], 1.0 / WSC
                        )
                        for mp in range(4):
                            qps = ps_s.tile([128, 2, 512], F32, tag="s")
                            for h2 in range(2):
                                m = 2 * mp + h2
                                cols = slice(128 * m, 128 * (m + 1))
                                i = 0
                                for lh, rh in dr_terms(
                                    wq8_sb, wqlo_sb, xq8_t, xqlo_t
                                ):
                                    for cp in range(NCP):
                                        nc.tensor.matmul(
                                            qps[:, h2, :],
                                            lh[:, cp, :, cols],
                                            rh[:, cp, :, :],
                                            start=(i == 0),
                                            stop=False,
                                            perf_mode=DR,
                                        )
                                        i += 1
                                nc.tensor.matmul(
                                    qps[:, h2, :],
                                    lb_qk_sb[:, cols],
                                    tmq_sb[:],
                                    start=False,
                                    stop=True,
                                )
                            nc.scalar.mul(
                                qT_sb[:, 2 * mp : 2 * mp + 2, sl], qps[:], 1.0 / WSC
                            )

                # ---- K^T and V per seq block ----
                def emit_kv_block(s, stage_on_act):
                    sl = slice(s * 512, (s + 1) * 512)
                    x8_t = xa_pool.tile([128, NCP, 2, 512], F8E4, tag="x8")
                    xlo_t = xa_pool.tile([128, NCP, 2, 512], F8E4, tag="xlo")
                    nc.sync.dma_start(out=x8_t[:], in_=x8[:, :, :, sl])
                    nc.sync.dma_start(out=xlo_t[:], in_=xlo[:, :, :, sl])
                    tsl = None
                    if s >= 2:  # e2 rows: attn lora mid
                        tsl = slice((s - 2) * 512, (s - 1) * 512)
                        tmp_ps = ps_s.tile([128, 2, 512], F32, tag="s")
                        mid_group(tmp_ps[0:RPAD, 0, :], x8_t, xlo_t)
                        nc.vector.tensor_scalar_mul(
                            tmp_kv[:, tsl], tmp_ps[0:RANK, 0, :], 1.0 / WSC
                        )
                    for mp in range(4):  # kcol tile pairs
                        kps = ps_s.tile([128, 2, 512], F32, tag="s")
                        for h2 in range(2):
                            m = 2 * mp + h2
                            cols = slice(128 * m, 128 * (m + 1))
                            i = 0
                            for lh, rh in dr_terms(wk8_sb, wklo_sb, x8_t, xlo_t):
                                for cp in range(NCP):
                                    nc.tensor.matmul(
                                        kps[:, h2, :],
                                        lh[:, cp, :, cols],
                                        rh[:, cp, :, :],
                                        start=(i == 0),
                                        stop=(i == 3 * NCP - 1 and s < 2),
                                        perf_mode=DR,
                                    )
                                    i += 1
                            if s >= 2:
                                nc.tensor.matmul(
                                    kps[:, h2, :],
                                    lb_qk_sb[:, C + 128 * m : C + 128 * (m + 1)],
                                    tmp_kv[:, tsl],
                                    start=False,
                                    stop=True,
                                )
                        dst = ktsb[:, 2 * mp : 2 * mp + 2, sl]
                        if stage_on_act:
                            nc.scalar.mul(dst, kps[:], 1.0 / WSC)
                        else:
                            nc.vector.tensor_scalar_mul(dst, kps[:], 1.0 / WSC)
                    for st in range(4):  # V: 128-row seq tiles within block
                        ssl = slice(128 * st, 128 * (st + 1))
                        vps = ps_s.tile([128, 2, 512], F32, tag="s")
                        for vc in range(2):
                            vsl = slice(512 * vc, 512 * (vc + 1))
                            i = 0
                            for lh, rh in (
                                (x8_t, wv8_sb), (xlo_t, wv8_sb), (x8_t, wvlo_sb),
                            ):
                                for cp in range(NCP):
                                    nc.tensor.matmul(
                                        vps[:, vc, :],
                                        lh[:, cp, :, ssl],
                                        rh[:, cp, :, vsl],
                                        start=(i == 0),
                                        stop=(i == 3 * NCP - 1 and s < 2),
                                        perf_mode=DR,
                                    )
                                    i += 1
                            if s >= 2:
                                base = (s - 2) * 512 + 128 * st
                                nc.tensor.matmul(
                                    vps[:, vc, :],
                                    tmp_kv[:, base : base + 128],
                                    lb_v_sb[:, vsl],
                                    start=False,
                                    stop=True,
                                )
                        dst = vres[:, 4 * s + st, :, 0:D]
                        vsrc = vps[:].rearrange("p v (h d) -> p (v h) d", h=8)
                        if stage_on_act:
                            nc.scalar.mul(dst, vsrc, 1.0 / WSC)
                        else:
                            nc.vector.tensor_scalar_mul(dst, vsrc, 1.0 / WSC)

                emit_kv_block(0, True)
                emit_kv_block(1, True)
                emit_kv_block(2, True)
                emit_attention_qb(0)
                emit_kv_block(3, False)

            # ===== phase B: output projection + qb1 ===========================
            bpool = ctx.enter_context(tc.tile_pool(name="bpool", bufs=1))
            wproj_sb = bpool.tile([128, NCH, C], BF16)
            nc.sync.dma_start(out=wproj_sb[:], in_=wproj[:])
            ost_pool = ctx.enter_context(tc.tile_pool(name="ost", bufs=2))

            def emit_proj_qb(qb):
                while pending:
                    _emit_division(*pending.pop(0))
                qb_sl = slice(qb * 512, (qb + 1) * 512)
                tm2_ps = ps_s.tile([128, 2, 512], F32, tag="s")
                for ch in range(NCH):
                    nc.tensor.matmul(
                        tm2_ps[0:RANK, 0, :],
                        la_proj_sb[:, ch, :],
                        y_acc[:, ch, qb_sl],
                        start=(ch == 0),
                        stop=(ch == NCH - 1),
                    )
                tm2_sb = small.tile([RANK, 512], BF16, tag="tm2")
                nc.vector.tensor_copy(tm2_sb[:], tm2_ps[0:RANK, 0, :])
                for qs in range(4):
                    qrow = 512 * qb + 128 * qs
                    ops = ps_s.tile([128, 2, 512], F32, tag="s")
                    for co in range(2):
                        cos = slice(512 * co, 512 * (co + 1))
                        for ch in range(NCH):
                            nc.tensor.matmul(
                                ops[:, co, :],
                                y_acc[:, ch, qrow : qrow + 128],
                                wproj_sb[:, ch, cos],
                                start=(ch == 0),
                                stop=False,
                            )
                        nc.tensor.matmul(
                            ops[:, co, :],
                            tm2_sb[:, 128 * qs : 128 * (qs + 1)],
                            lb_proj_sb[:, cos],
                            start=False,
                            stop=True,
                        )
                    ost = ost_pool.tile([128, 2, 512], F32, tag="ost")
                    nc.vector.tensor_copy(ost[:], ops[:])
                    nc.sync.dma_start(
                        out=yout[qrow : qrow + 128, :],
                        in_=ost[:].rearrange("p a b -> p (a b)"),
                    )

            emit_proj_qb(0)
            emit_attention_qb(1)
            emit_proj_qb(1)
    return nc


_PROGRAM = None


def _get_program():
    global _PROGRAM
    if _PROGRAM is None:
        _PROGRAM = _build_program()
    return _PROGRAM


# ---------------------------------------------------------------------------
# Host side
# ---------------------------------------------------------------------------
def _delayed_mask_np(t):
    ones = np.ones((t, t), dtype=bool)
    m11 = np.tril(ones) & np.triu(ones, -(LOOKAHEAD + OVERLAP))
    m12 = np.tril(ones, -LOOKAHEAD)
    m21 = np.tril(ones, LOOKAHEAD) & np.triu(ones, -OVERLAP)
    m22 = np.tril(ones)
    return np.block([[m11, m12], [m21, m22]])


def _fp8_pair(a):
    hi = a.astype(FP8NP)
    lo = (a - hi.astype(np.float32)).astype(FP8NP)
    return hi, lo


def _cp_layout(m):
    """[C, N] -> [128, NCP, 2, N] with c = 256*cp + 128*i + p."""
    n = m.shape[1]
    return np.ascontiguousarray(m.reshape(NCP, 2, 128, n).transpose(2, 0, 1, 3))


def kernel(
    e1,
    e2,
    W_attn,
    W_proj,
    lora_A_attn,
    lora_B_attn,
    lora_A_proj,
    lora_B_proj,
    _trace=False,
):
    f32 = np.float32
    bf16 = ml_dtypes.bfloat16
    e1 = np.asarray(e1, f32)
    e2 = np.asarray(e2, f32)
    W_attn = np.asarray(W_attn, f32)
    W_proj = np.asarray(W_proj, f32)
    lora_A_attn = np.asarray(lora_A_attn, f32)
    lora_B_attn = np.asarray(lora_B_attn, f32)
    lora_A_proj = np.asarray(lora_A_proj, f32)
    lora_B_proj = np.asarray(lora_B_proj, f32)
    nc = _get_program()
    M = _delayed_mask_np(T)

    # --- role-independent prep (once) ---
    wq8, wqlo = _fp8_pair(_cp_layout(W_attn[:, :C] * (WSC * QSCALE)))
    wk8, wklo = _fp8_pair(_cp_layout(W_attn[:, C : 2 * C] * WSC))
    wv8, wvlo = _fp8_pair(_cp_layout(W_attn[:, 2 * C :] * WSC))
    la_pad = np.zeros((C, RPAD), f32)
    la_pad[:, :RANK] = lora_A_attn * WSC
    la8, lalo = _fp8_pair(_cp_layout(la_pad))
    la_proj = np.ascontiguousarray(
        lora_A_proj.reshape(NCH, 128, RANK).transpose(1, 0, 2)
    ).astype(bf16)
    wproj_r = np.ascontiguousarray(
        W_proj.reshape(NCH, 128, C).transpose(1, 0, 2)
    ).astype(bf16)
    lb_v = (np.ascontiguousarray(lora_B_attn[:, 2 * C :]) * (LSCALE * WSC)).astype(
        bf16
    )
    ones1 = np.ones((1, 128), f32)

    lbqk = {}
    lbp = {}
    for r in (0, 1):
        q = np.array(lora_B_attn[:, :C], dtype=f32) * (LSCALE * WSC * QSCALE)
        if r == 0:
            q[:] = 0.0
        k = lora_B_attn[:, C : 2 * C] * (LSCALE * WSC)
        lbqk[r] = np.concatenate([q, k], axis=1).astype(bf16)
        p = np.array(lora_B_proj, dtype=f32) * LSCALE
        if r == 0:
            p[:] = 0.0
        lbp[r] = p.astype(bf16)

    masks_r = {}
    for r in (0, 1):
        mk = np.empty((NMASK, 128, 128), dtype=bf16)
        for i, (region, j, qt) in enumerate(MASK_TILES):
            qg = r * T + 128 * qt
            kg = (0 if region == "e1" else T) + 128 * j
            mk[i] = M[qg : qg + 128, kg : kg + 128].T.astype(f32)
        masks_r[r] = mk

    in_maps = []
    x_cache = None
    for core in range(8):
        b, r = core // 2, core % 2
        if r == 0:
            x = np.concatenate([e1[b], e2[b]], axis=0)  # [2T, C]
            xT = np.ascontiguousarray(x.T)
            x_cache = _fp8_pair(_cp_layout(xT))
        x8b, xlob = x_cache
        qsl = slice(r * T, (r + 1) * T)
        in_maps.append({
            "x8": x8b,
            "xlo": xlob,
            "xq8": np.ascontiguousarray(x8b[:, :, :, qsl]),
            "xqlo": np.ascontiguousarray(xlob[:, :, :, qsl]),
            "wq8": wq8, "wqlo": wqlo,
            "wk8": wk8, "wklo": wklo,
            "wv8": wv8, "wvlo": wvlo,
            "la8": la8, "lalo": lalo,
            "lb_qk": lbqk[r],
            "lb_v": lb_v,
            "la_proj": la_proj,
            "lb_proj": lbp[r],
            "wproj": wproj_r,
            "masks": masks_r[r],
            "ones1": ones1,
        })

    res = run_bass_kernel_spmd(nc, in_maps, core_ids=list(range(8)), trace=_trace)
    y1 = np.stack([res.results[2 * b]["yout"] for b in range(B)])
    y2 = np.stack([res.results[2 * b + 1]["yout"] for b in range(B)])
    if _trace:
        kernel.last_results = res
    return y1, y2


# revision 52
# speedup vs baseline: 1.2376x; 1.0073x over previous
"""Trainium2 Bass kernel for nn_DelayedSelfAttention (B=4, T=1024, C=1024, H=16).

Sharding: 8 cores = 4 batches x 2 sequence-halves.  Core c handles batch
c//2 and query rows [r*T, (r+1)*T) of the concatenated [2T] sequence
(r = c%2).  Each core computes K/V for the full 2T sequence (duplicated
kv-projection -- cheaper than any collective on this fabric), attention
for its T query rows over all 16 heads, and the output projection for
its rows.  Role asymmetry (mask values, q/proj LoRA) is pushed into
per-core input data so a single SPMD program serves all cores.

v2 vs the spill-to-DRAM baseline:
 - K^T and V stay RESIDENT in SBUF (no DRAM spill + reload).
 - QKV projections run as compensated fp8e4m3 DoubleRow matmuls:
   x ~ x8 + xlo, W ~ W8 + Wlo (host-quantized; weights prescaled by 64
   to clear the e4m3 subnormal range, staging copies scale by 1/64).
   Three DR terms (x8W8 + xloW8 + x8Wlo) cover a 256-deep contraction
   in 1.5 row-passes vs bf16's 2 -- ~25% tensor-engine saving at ~0.25%
   error (compensation cancels first-order quantization).
 - exp batched per head-pair ([128, 2, nq] PSUM duos), masks multiplied
   with a stride-0 head-broadcast, head-phase staging copies on the
   (otherwise idle) scalar engine.
 - emission order software-pipelines the phases: Q-proj, K/V blocks
   s0..s2, q-block-0 attention overlapping the s3 projection, then the
   qb0 output projection, qb1 attention, qb1 projection.
"""

import contextlib
import sys

for _p in ("/opt/trn_rl_repo", "/root/.axon_site/_ro/trn_rl_repo"):
    if _p not in sys.path:
        sys.path.insert(0, _p)

import ml_dtypes
import numpy as np

import concourse.bass as bass
import concourse.mybir as mybir
import concourse.tile as tile_mod
from concourse.bass_utils import run_bass_kernel_spmd
from concourse.tile import TileContext
from concourse.vector_clock import ScopedClock

# ---------------------------------------------------------------------------
# Workaround: this walrus build supports a single semaphore wait per
# instruction.  Split multi-wait instructions into same-engine NoOps each
# carrying one wait (identical sequencer semantics).
# ---------------------------------------------------------------------------
_ws_counter = [0]


def _fresh_name():
    _ws_counter[0] += 1
    return f"I-waitsplit-{_ws_counter[0]}"


def _split_inst_waits(inst):
    si = inst.sync_info
    if si is None:
        return []
    waits = list(si.on_wait or [])
    if len(waits) <= 1:
        return []
    nops = []
    for w in waits[:-1]:
        nop = mybir.InstNoOp(name=_fresh_name())
        nop.engine = inst.engine
        nop.sync_info = mybir.SyncInfo(on_wait=[w], on_update=[])
        nops.append(nop)
    inst.sync_info = mybir.SyncInfo(
        on_wait=[waits[-1]], on_update=list(si.on_update or [])
    )
    return nops


_orig_lower = tile_mod.TileContext._lower_ordered_insts


def _patched_lower(self, ordered):
    for bb_name in list(ordered.keys()):
        new = []
        for inst in ordered[bb_name]:
            new.extend(_split_inst_waits(inst))
            new.append(inst)
        ordered[bb_name] = new
    return _orig_lower(self, ordered)


def _patched_drain_and_barrier(self, tick_clock, wait_clock):
    nc = self.nc
    drain_inst = nc.sync.drain()
    wait_clock.add_sem_waits(
        drain_inst.ins, ScopedClock({None: tick_clock.global_clock})
    )
    nops = _split_inst_waits(drain_inst.ins)
    if nops:
        first_wait = drain_inst.ins.sync_info
        drain_inst.ins.sync_info = mybir.SyncInfo(on_wait=[], on_update=[])
        for nop in nops:
            n2 = nc.sync.nop(nofuse=True)
            n2.ins.sync_info = nop.sync_info
        d2 = nc.sync.drain()
        d2.ins.sync_info = first_wait

    nc.all_engine_barrier()
    assert self.sems is not None
    popped = nc._tile_sem_poison_stack.pop()
    assert popped is self._sem_poison
    nc.clear_and_free_semaphores(list(self.sems.allocated().values()))
    nc.all_engine_barrier()


def _apply_tile_patch():
    if tile_mod.TileContext._lower_ordered_insts is not _patched_lower:
        tile_mod.TileContext._lower_ordered_insts = _patched_lower
        tile_mod.TileContext._drain_and_barrier = _patched_drain_and_barrier


# ---------------------------------------------------------------------------
# Problem constants (hardcoded per the task contract).
# ---------------------------------------------------------------------------
B, T, C, H = 4, 1024, 1024, 16
D = C // H  # 64
SEQ = 2 * T
LOOKAHEAD, OVERLAP = 64, 64
RANK, ALPHA = 8, 16.0
RPAD = 16  # lora-A stationary padded (dual-fp8 ldweights needs width >= 16)
LSCALE = ALPHA / RANK  # 2.0
QSCALE = 1.0 / np.sqrt(D)  # 1/8
WSC = 64.0  # fp8 weight prescale (cleared by 1/WSC at staging)
NCH = C // 128  # 8 c-chunks
NCP = NCH // 2  # 4 c-chunk-pairs (DoubleRow)
NQT = T // 128  # 8 q-subtiles per core
F32 = mybir.dt.float32
F32R = mybir.dt.float32r
BF16 = mybir.dt.bfloat16
F8E4 = mybir.dt.float8e4
FP8NP = ml_dtypes.float8_e4m3fn
DR = mybir.MatmulPerfMode.DoubleRow


# Trace-time tiling structure, shared by host (mask packing) and device.
def _ktiles_for_qblock(qb):
    """k-tiles (region, j) touched by q-subtiles [4qb, 4qb+4)."""
    qts = range(4 * qb, 4 * qb + 4)
    e1 = sorted({j for qt in qts for j in (qt - 1, qt, qt + 1) if 0 <= j < NQT})
    e2 = sorted({j for qt in qts for j in range(qt + 1)})
    return [("e1", j) for j in e1] + [("e2", j) for j in e2]


def _active_qts(region, j, qb):
    if region == "e1":
        qts = [qt for qt in range(4 * qb, 4 * qb + 4) if j in (qt - 1, qt, qt + 1)]
    else:
        qts = [qt for qt in range(4 * qb, 4 * qb + 4) if j <= qt]
    assert qts == list(range(qts[0], qts[-1] + 1))
    return qts


def _mask_tiles():
    out = []
    for qt in range(NQT):
        for j in (qt - 1, qt, qt + 1):
            if 0 <= j < NQT:
                out.append(("e1", j, qt))
        for j in (qt - 1, qt):
            if j >= 0:
                out.append(("e2", j, qt))
    return out


MASK_TILES = _mask_tiles()  # 37 tiles
MASK_IDX = {k: i for i, k in enumerate(MASK_TILES)}
NMASK = len(MASK_TILES)


# ---------------------------------------------------------------------------
# Device program
# ---------------------------------------------------------------------------
def _build_program():
    _apply_tile_patch()
    nc = bass.Bass("TRN2", target_bir_lowering=False, debug=False, num_devices=8)

    def din(name, shape, dt=F32R):
        return nc.dram_tensor(name, list(shape), dt, kind="ExternalInput").ap()

    x8 = din("x8", (128, NCP, 2, SEQ), dt=F8E4)
    xlo = din("xlo", (128, NCP, 2, SEQ), dt=F8E4)
    xq8 = din("xq8", (128, NCP, 2, T), dt=F8E4)
    xqlo = din("xqlo", (128, NCP, 2, T), dt=F8E4)
    wq8 = din("wq8", (128, NCP, 2, C), dt=F8E4)
    wqlo = din("wqlo", (128, NCP, 2, C), dt=F8E4)
    wk8 = din("wk8", (128, NCP, 2, C), dt=F8E4)
    wklo = din("wklo", (128, NCP, 2, C), dt=F8E4)
    wv8 = din("wv8", (128, NCP, 2, C), dt=F8E4)
    wvlo = din("wvlo", (128, NCP, 2, C), dt=F8E4)
    la8 = din("la8", (128, NCP, 2, RPAD), dt=F8E4)
    lalo = din("lalo", (128, NCP, 2, RPAD), dt=F8E4)
    lb_qk = din("lb_qk", (RANK, 2 * C), dt=BF16)  # scaled, role-zeroed q
    lb_v = din("lb_v", (RANK, C), dt=BF16)
    la_proj = din("la_proj", (128, NCH, RANK), dt=BF16)
    lb_proj = din("lb_proj", (RANK, C), dt=BF16)  # zeroed for role 0
    wproj = din("wproj", (128, NCH, C), dt=BF16)
    masks = din("masks", (NMASK, 128, 128), dt=BF16)
    ones1 = din("ones1", (1, 128))
    yout = nc.dram_tensor("yout", [T, C], F32, kind="ExternalOutput").ap()

    with TileContext(nc) as tc:
        ctx = contextlib.ExitStack()
        with ctx:
            ctx.enter_context(
                nc.allow_low_precision(reason="float32r is full-width fp32 storage")
            )
            # --- persistent SBUF ---
            persist = ctx.enter_context(tc.tile_pool(name="persist", bufs=1))
            ktsb = persist.tile([128, NCH, SEQ], BF16)      # resident K^T
            vres = persist.tile([128, 16, H, D + 1], BF16)  # resident V + ones col
            qT_sb = persist.tile([128, NCH, T], BF16)       # resident Q^T (prescaled)
            y_acc = persist.tile([128, NCH, T], BF16)       # normalized y
            mask_sb = persist.tile([128, NMASK, 128], BF16)
            tmp_kv = persist.tile([RANK, T], BF16)          # e2 attn-lora mid
            la8_sb = persist.tile([128, NCP, 2, RPAD], F8E4)
            lalo_sb = persist.tile([128, NCP, 2, RPAD], F8E4)
            lb_qk_sb = persist.tile([RANK, 2 * C], BF16)
            lb_v_sb = persist.tile([RANK, C], BF16)
            la_proj_sb = persist.tile([128, NCH, RANK], BF16)
            lb_proj_sb = persist.tile([RANK, C], BF16)
            ones1_sb = persist.tile([1, 128], F32R)

            nc.vector.memset(vres[:, :, :, D : D + 1], 1.0)  # ones column

            # --- PSUM pools: 4 (yu / r_bc) + 2x2 (score duos) = 8 banks
            ps_y = ctx.enter_context(tc.tile_pool(name="ps_y", bufs=4, space="PSUM"))
            ps_s = ctx.enter_context(tc.tile_pool(name="ps_s", bufs=2, space="PSUM"))

            small = ctx.enter_context(tc.tile_pool(name="small", bufs=1))
            pt_pool = ctx.enter_context(tc.tile_pool(name="pt", bufs=10))
            ysb_pool = ctx.enter_context(tc.tile_pool(name="ysb", bufs=3))

            pending = []
            holders = {}

            def _division_stage1(dyus):
                ysbs = []
                for hi in range(2):
                    ysb = ysb_pool.tile([D + 1, 512], F32R, tag="ysb")
                    nc.vector.tensor_copy(ysb[:], dyus[hi][:])
                    r_tmp = ysb_pool.tile([1, 512], F32R, tag="rt")
                    nc.vector.reciprocal(r_tmp[:], ysb[D : D + 1, :])
                    ysbs.append((ysb, r_tmp))
                return ysbs

            def _division_stage2(dqb, dp, ysbs):
                dqb_sl = slice(dqb * 512, (dqb + 1) * 512)
                y_acc = holders["y_acc"]
                for hi in range(2):
                    ysb, r_tmp = ysbs[hi]
                    r_bc = ps_y.tile([128, 512], F32, tag="y")
                    nc.tensor.matmul(
                        r_bc[:], ones1_sb[:], r_tmp[:],
                        start=True, stop=True,
                    )
                    rows = slice(64 * hi, 64 * hi + 64)
                    nc.vector.tensor_mul(
                        y_acc[rows, dp, dqb_sl], ysb[0:D, :], r_bc[rows, :]
                    )

            def emit_attention_p(qb, p, bracket=()):
                """Attention for q-block qb, c-chunk p (head pair 2p, 2p+1).

                AV matmuls lag scores by one k-tile so the tensor engine has
                work while exp/mask chains drain; bracket thunks (projection
                work) are popped one per k-tile to fill remaining gaps.
                """
                items = list(bracket)
                ktl = _ktiles_for_qblock(qb)
                yus = [
                    ps_y.tile([D + 1, 512], F32, tag="y", name=f"yu_{qb}_{p}_{i}")
                    for i in range(2)
                ]
                avq = []

                def flush_av():
                    ki0, pt0, nq0, rel0, st0 = avq.pop(0)
                    for hi in range(2):
                        nc.tensor.matmul(
                            yus[hi][:, rel0],
                            vres[:, st0, 2 * p + hi, :],
                            pt0[:, hi, 0:nq0],
                            start=(ki0 == 0),
                            stop=(ki0 == len(ktl) - 1),
                            skip_group_check=True,
                        )

                for ki, (region, j) in enumerate(ktl):
                    qts = _active_qts(region, j, qb)
                    qlo, qw = qts[0], len(qts)
                    q_sl = slice(128 * qlo, 128 * (qlo + qw))
                    rel_sl = slice(128 * (qlo - 4 * qb), 128 * (qlo - 4 * qb + qw))
                    nq = 128 * qw
                    kbase = (0 if region == "e1" else T) + 128 * j
                    st_glob = kbase // 128

                    if ki == 2 and pending:
                        _division_stage2(*pending.pop(0))
                    if items and ki >= 1:
                        items.pop(0)()

                    sp = ps_s.tile([128, 2, 512], F32, tag="s")
                    for hi in range(2):
                        lo = 64 * hi
                        nc.tensor.matmul(
                            sp[:, hi, 0:nq],
                            ktsb[lo : lo + 64, p, kbase : kbase + 128],
                            qT_sb[lo : lo + 64, p, q_sl],
                            start=True,
                            stop=True,
                        )
                    pt = pt_pool.tile([128, 2, 512], BF16, tag="pt")
                    nc.scalar.activation(
                        pt[:, :, 0:nq],
                        sp[:, :, 0:nq],
                        mybir.ActivationFunctionType.Exp,
                    )
                    for qt in qts:
                        if (region, j, qt) in MASK_IDX:
                            mi = MASK_IDX[(region, j, qt)]
                            rel = slice(128 * (qt - qlo), 128 * (qt - qlo + 1))
                            mb = mask_sb[:, mi : mi + 1, :].broadcast_to(
                                [128, 2, 128]
                            )
                            nc.vector.tensor_mul(pt[:, :, rel], pt[:, :, rel], mb)
                    avq.append((ki, pt, nq, rel_sl, st_glob))
                    flush_av()
                while avq:
                    flush_av()
                for it in items:
                    it()
                pending.append((qb, p, _division_stage1(yus)))

            # ===== phase A: projections (compensated fp8 DoubleRow) ============
            def mid_group(out_ap, x8_t, xlo_t, sl):
                """attn-lora mid: sum_c A[c, :]^T x[c, sl] -> [RPAD, 512]."""
                i = 0
                for cp in range(NCP):
                    for lh, rh in (
                        (la8_sb, x8_t), (lalo_sb, x8_t), (la8_sb, xlo_t),
                    ):
                        nc.tensor.matmul(
                            out_ap,
                            lh[:, cp, :, :],
                            rh[:, cp, :, sl],
                            start=(i == 0),
                            stop=(i == 3 * NCP - 1),
                            perf_mode=DR,
                        )
                        i += 1

            actx = contextlib.ExitStack()
            with actx:
                wk_pool = actx.enter_context(tc.tile_pool(name="wk", bufs=1))
                xa_pool = actx.enter_context(tc.tile_pool(name="xa", bufs=2))

                def load_xq(s):
                    sl = slice(s * 512, (s + 1) * 512)
                    xq8_t = xa_pool.tile([128, NCP, 2, 512], F8E4, tag="x8",
                                         name=f"xq8_{s}")
                    xqlo_t = xa_pool.tile([128, NCP, 2, 512], F8E4, tag="xlo",
                                          name=f"xqlo_{s}")
                    nc.sync.dma_start(out=xq8_t[:], in_=xq8[:, :, :, sl])
                    nc.sync.dma_start(out=xqlo_t[:], in_=xqlo[:, :, :, sl])
                    return xq8_t, xqlo_t

                # ---- Q^T projection first (own T rows), resident ----
                with tc.tile_pool(name="wq", bufs=1) as wq_pool:
                    wq8_sb = wq_pool.tile([128, NCP, 2, C], F8E4)
                    wqlo_sb = wq_pool.tile([128, NCP, 2, C], F8E4)
                    nc.sync.dma_start(out=wq8_sb[:], in_=wq8[:])
                    xqt = {s_: load_xq(s_) for s_ in range(2)}
                    nc.sync.dma_start(out=la8_sb[:], in_=la8[:])
                    nc.sync.dma_start(out=lalo_sb[:], in_=lalo[:])
                    nc.sync.dma_start(out=wqlo_sb[:], in_=wqlo[:])
                    nc.sync.dma_start(out=lb_qk_sb[:], in_=lb_qk[:])
                    # prefetch K weights while Q computes
                    wk8_sb = wk_pool.tile([128, NCP, 2, C], F8E4)
                    wklo_sb = wk_pool.tile([128, NCP, 2, C], F8E4)
                    nc.sync.dma_start(out=wk8_sb[:], in_=wk8[:])
                    nc.sync.dma_start(out=wklo_sb[:], in_=wklo[:])
                    nc.sync.dma_start(
                        out=mask_sb[:], in_=masks.rearrange("t p q -> p t q")
                    )
                    nc.sync.dma_start(out=lb_v_sb[:], in_=lb_v[:])
                    nc.sync.dma_start(out=la_proj_sb[:], in_=la_proj[:])
                    nc.sync.dma_start(out=lb_proj_sb[:], in_=lb_proj[:])
                    nc.sync.dma_start(out=ones1_sb[:], in_=ones1[:])

                    for s_ in range(2):
                        sl = slice(s_ * 512, (s_ + 1) * 512)
                        xq8_t, xqlo_t = xqt[s_]
                        tmq_ps = ps_s.tile([128, 2, 512], F32, tag="s")
                        mid_group(tmq_ps[0:RPAD, 0, :], xq8_t, xqlo_t,
                                  slice(0, 512))
                        tmq_sb = small.tile([RANK, 512], BF16, tag="tmq")
                        nc.vector.tensor_scalar_mul(
                            tmq_sb[:], tmq_ps[0:RANK, 0, :], 1.0 / WSC
                        )
                        for mp in range(4):
                            qps = ps_s.tile([128, 2, 512], F32, tag="s")
                            for h2 in range(2):
                                m = 2 * mp + h2
                                cols = slice(128 * m, 128 * (m + 1))
                                i = 0
                                for lh, rh in (
                                    (wq8_sb, xq8_t),
                                    (wqlo_sb, xq8_t),
                                    (wq8_sb, xqlo_t),
                                ):
                                    for cp in range(NCP):
                                        nc.tensor.matmul(
                                            qps[:, h2, :],
                                            lh[:, cp, :, cols],
                                            rh[:, cp, :, :],
                                            start=(i == 0),
                                            stop=False,
                                            perf_mode=DR,
                                        )
                                        i += 1
                                nc.tensor.matmul(
                                    qps[:, h2, :],
                                    lb_qk_sb[:, cols],
                                    tmq_sb[:],
                                    start=False,
                                    stop=True,
                                )
                            nc.scalar.mul(
                                qT_sb[:, 2 * mp : 2 * mp + 2, sl], qps[:], 1.0 / WSC
                            )

                wv_pool = actx.enter_context(tc.tile_pool(name="wv", bufs=1))
                wv8_sb = wv_pool.tile([128, NCP, 2, C], F8E4)
                wvlo_sb = wv_pool.tile([128, NCP, 2, C], F8E4)
                nc.sync.dma_start(out=wv8_sb[:], in_=wv8[:])
                nc.sync.dma_start(out=wvlo_sb[:], in_=wvlo[:])
                holders["y_acc"] = y_acc

                def load_x(s):
                    sl = slice(s * 512, (s + 1) * 512)
                    x8_t = xa_pool.tile([128, NCP, 2, 512], F8E4, tag="x8",
                                        name=f"x8_{s}")
                    xlo_t = xa_pool.tile([128, NCP, 2, 512], F8E4, tag="xlo",
                                         name=f"xlo_{s}")
                    nc.sync.dma_start(out=x8_t[:], in_=x8[:, :, :, sl])
                    nc.sync.dma_start(out=xlo_t[:], in_=xlo[:, :, :, sl])
                    return x8_t, xlo_t

                def emit_mid(s, x8_t, xlo_t):
                    tsl = slice((s - 2) * 512, (s - 1) * 512)
                    tmp_ps = ps_s.tile([128, 2, 512], F32, tag="s")
                    mid_group(tmp_ps[0:RPAD, 0, :], x8_t, xlo_t, slice(0, 512))
                    nc.vector.tensor_scalar_mul(
                        tmp_kv[:, tsl], tmp_ps[0:RANK, 0, :], 1.0 / WSC
                    )

                def emit_k_duo(s, mp, x8_t, xlo_t, stage_on_act=True):
                    sl = slice(s * 512, (s + 1) * 512)
                    tsl = slice((s - 2) * 512, (s - 1) * 512) if s >= 2 else None
                    kps = ps_s.tile([128, 2, 512], F32, tag="s")
                    for h2 in range(2):
                        m = 2 * mp + h2
                        cols = slice(128 * m, 128 * (m + 1))
                        i = 0
                        for lh, rh in (
                            (wk8_sb, x8_t), (wklo_sb, x8_t), (wk8_sb, xlo_t),
                        ):
                            for cp in range(NCP):
                                nc.tensor.matmul(
                                    kps[:, h2, :],
                                    lh[:, cp, :, cols],
                                    rh[:, cp, :, :],
                                    start=(i == 0),
                                    stop=(i == 3 * NCP - 1 and s < 2),
                                    perf_mode=DR,
                                )
                                i += 1
                        if s >= 2:
                            nc.tensor.matmul(
                                kps[:, h2, :],
                                lb_qk_sb[:, C + 128 * m : C + 128 * (m + 1)],
                                tmp_kv[:, tsl],
                                start=False,
                                stop=True,
                            )
                    dst = ktsb[:, 2 * mp : 2 * mp + 2, sl]
                    if stage_on_act:
                        nc.scalar.mul(dst, kps[:], 1.0 / WSC)
                    else:
                        nc.vector.tensor_scalar_mul(dst, kps[:], 1.0 / WSC)

                def emit_v_duo(s, st, x8_t, xlo_t, stage_on_act=True):
                    ssl = slice(128 * st, 128 * (st + 1))
                    vps = ps_s.tile([128, 2, 512], F32, tag="s")
                    for vc in range(2):
                        vsl = slice(512 * vc, 512 * (vc + 1))
                        i = 0
                        for lh, rh in (
                            (x8_t, wv8_sb), (xlo_t, wv8_sb), (x8_t, wvlo_sb),
                        ):
                            for cp in range(NCP):
                                nc.tensor.matmul(
                                    vps[:, vc, :],
                                    lh[:, cp, :, ssl],
                                    rh[:, cp, :, vsl],
                                    start=(i == 0),
                                    stop=(i == 3 * NCP - 1 and s < 2),
                                    perf_mode=DR,
                                )
                                i += 1
                        if s >= 2:
                            base = (s - 2) * 512 + 128 * st
                            nc.tensor.matmul(
                                vps[:, vc, :],
                                tmp_kv[:, base : base + 128],
                                lb_v_sb[:, vsl],
                                start=False,
                                stop=True,
                            )
                    dst = vres[:, 4 * s + st, :, 0:D]
                    vsrc = vps[:].rearrange("p v (h d) -> p (v h) d", h=8)
                    if stage_on_act:
                        nc.scalar.mul(dst, vsrc, 1.0 / WSC)
                    else:
                        nc.vector.tensor_scalar_mul(dst, vsrc, 1.0 / WSC)

                def emit_v_single(s, st, vc, x8_t, xlo_t, stage_on_act=True):
                    ssl = slice(128 * st, 128 * (st + 1))
                    vsl = slice(512 * vc, 512 * (vc + 1))
                    vps = ps_s.tile([128, 2, 512], F32, tag="s")
                    i = 0
                    for lh, rh in (
                        (x8_t, wv8_sb), (xlo_t, wv8_sb), (x8_t, wvlo_sb),
                    ):
                        for cp in range(NCP):
                            nc.tensor.matmul(
                                vps[:, 0, :],
                                lh[:, cp, :, ssl],
                                rh[:, cp, :, vsl],
                                start=(i == 0),
                                stop=(i == 3 * NCP - 1 and s < 2),
                                perf_mode=DR,
                            )
                            i += 1
                    if s >= 2:
                        base = (s - 2) * 512 + 128 * st
                        nc.tensor.matmul(
                            vps[:, 0, :],
                            tmp_kv[:, base : base + 128],
                            lb_v_sb[:, vsl],
                            start=False,
                            stop=True,
                        )
                    dst = vres[:, 4 * s + st, 8 * vc : 8 * vc + 8, 0:D]
                    vsrc = vps[:, 0, :].rearrange("p (h d) -> p h d", h=8)
                    if stage_on_act:
                        nc.scalar.mul(dst, vsrc, 1.0 / WSC)
                    else:
                        nc.vector.tensor_scalar_mul(dst, vsrc, 1.0 / WSC)

                # ---- blocks s0..s2 up front; s3 is emitted inside qb0
                # attention windows to keep the tensor engine fed ----
                for s_ in range(3):
                    x8_t, xlo_t = load_x(s_)
                    if s_ >= 2:
                        emit_mid(s_, x8_t, xlo_t)
                    for mp in range(4):
                        emit_k_duo(s_, mp, x8_t, xlo_t)
                    for st in range(4):
                        emit_v_duo(s_, st, x8_t, xlo_t)

                s3x = {}

                def s3_item(kind, idx):
                    def run():
                        if "x" not in s3x:
                            s3x["x"] = load_x(3)
                            emit_mid(3, *s3x["x"])
                        x8_t, xlo_t = s3x["x"]
                        if kind == "k":
                            emit_k_duo(3, idx, x8_t, xlo_t, stage_on_act=False)
                        else:
                            emit_v_duo(3, idx, x8_t, xlo_t, stage_on_act=False)
                    return run

                brackets0 = {
                    0: [s3_item("k", 0)],
                    1: [s3_item("v", 0)],
                    2: [s3_item("k", 1)],
                    3: [s3_item("v", 1)],
                    4: [s3_item("k", 2)],
                    5: [s3_item("v", 2)],
                    6: [s3_item("k", 3)],
                    7: [s3_item("v", 3)],
                }
                for p in range(8):
                    emit_attention_p(0, p, bracket=brackets0.get(p, ()))

            # ===== phase B: output projection + qb1 ===========================
            bpool = ctx.enter_context(tc.tile_pool(name="bpool", bufs=1))
            wproj_sb = bpool.tile([128, NCH, C], BF16)
            nc.sync.dma_start(out=wproj_sb[:], in_=wproj[:])
            ost_pool = ctx.enter_context(tc.tile_pool(name="ost", bufs=2))

            proj_state = {}

            def proj_tm2(qb):
                qb_sl = slice(qb * 512, (qb + 1) * 512)
                tm2_ps = ps_s.tile([128, 2, 512], F32, tag="s")
                for ch in range(NCH):
                    nc.tensor.matmul(
                        tm2_ps[0:RANK, 0, :],
                        la_proj_sb[:, ch, :],
                        y_acc[:, ch, qb_sl],
                        start=(ch == 0),
                        stop=(ch == NCH - 1),
                    )
                tm2_sb = small.tile([RANK, 512], BF16, tag="tm2")
                nc.vector.tensor_copy(tm2_sb[:], tm2_ps[0:RANK, 0, :])
                proj_state[qb] = tm2_sb

            def proj_qs(qb, qs):
                tm2_sb = proj_state[qb]
                qrow = 512 * qb + 128 * qs
                ops = ps_s.tile([128, 2, 512], F32, tag="s")
                for co in range(2):
                    cos = slice(512 * co, 512 * (co + 1))
                    for ch in range(NCH):
                        nc.tensor.matmul(
                            ops[:, co, :],
                            y_acc[:, ch, qrow : qrow + 128],
                            wproj_sb[:, ch, cos],
                            start=(ch == 0),
                            stop=False,
                        )
                    nc.tensor.matmul(
                        ops[:, co, :],
                        tm2_sb[:, 128 * qs : 128 * (qs + 1)],
                        lb_proj_sb[:, cos],
                        start=False,
                        stop=True,
                    )
                ost = ost_pool.tile([128, 2, 512], F32, tag="ost")
                nc.vector.tensor_copy(ost[:], ops[:])
                nc.sync.dma_start(
                    out=yout[qrow : qrow + 128, :],
                    in_=ost[:].rearrange("p a b -> p (a b)"),
                )

            proj0 = [lambda: proj_tm2(0)] + [
                (lambda qs=qs: proj_qs(0, qs)) for qs in range(4)
            ]
            brackets1 = {1: proj0[0:2], 2: proj0[2:3], 3: proj0[3:4], 4: proj0[4:5]}
            for p in range(8):
                emit_attention_p(1, p, bracket=brackets1.get(p, ()))

            while pending:
                _division_stage2(*pending.pop(0))
            proj_tm2(1)
            for qs in range(4):
                proj_qs(1, qs)
    return nc.dram_tensor(name, list(shape), dt, kind="ExternalInput").ap()

    x8 = din("x8", (128, NCP, 2, SEQ), dt=F8E4)
    xlo = din("xlo", (128, NCP, 2, SEQ), dt=F8E4)
    xq8 = din("xq8", (128, NCP, 2, T), dt=F8E4)
    xqlo = din("xqlo", (128, NCP, 2, T), dt=F8E4)
    wq8 = din("wq8", (128, NCP, 2, C), dt=F8E4)
    wqlo = din("wqlo", (128, NCP, 2, C), dt=F8E4)
    wk8 = din("wk8", (128, NCP, 2, C), dt=F8E4)
    wklo = din("wklo", (128, NCP, 2, C), dt=F8E4)
    wv8 = din("wv8", (128, NCP, 2, C), dt=F8E4)
    wvlo = din("wvlo", (128, NCP, 2, C), dt=F8E4)
    la8 = din("la8", (128, NCP, 2, RPAD), dt=F8E4)
    lalo = din("lalo", (128, NCP, 2, RPAD), dt=F8E4)
    lb_qk = din("lb_qk", (RANK, 2 * C), dt=BF16)  # scaled, role-zeroed q
    lb_v = din("lb_v", (RANK, C), dt=BF16)
    la_proj = din("la_proj", (128, NCH, RANK), dt=BF16)
    lb_proj = din("lb_proj", (RANK, C), dt=BF16)  # zeroed for role 0
    wproj = din("wproj", (128, NCH, C), dt=BF16)
    masks = din("masks", (NMASK, 128, 128), dt=BF16)
    ones1 = din("ones1", (1, 128))
    yout = nc.dram_tensor("yout", [T, C], F32, kind="ExternalOutput").ap()

    with TileContext(nc) as tc:
        ctx = contextlib.ExitStack()
        with ctx:
            ctx.enter_context(
                nc.allow_low_precision(reason="float32r is full-width fp32 storage")
            )
            # --- persistent SBUF ---
            persist = ctx.enter_context(tc.tile_pool(name="persist", bufs=1))
            ktsb = persist.tile([128, NCH, SEQ], BF16)      # resident K^T
            vres = persist.tile([128, 16, H, D + 1], BF16)  # resident V + ones col
            qT_sb = persist.tile([128, NCH, T], BF16)       # resident Q^T (prescaled)
            y_acc = persist.tile([128, NCH, T], BF16)       # normalized y
            mask_sb = persist.tile([128, NMASK, 128], BF16)
            tmp_kv = persist.tile([RANK, T], BF16)          # e2 attn-lora mid
            la8_sb = persist.tile([128, NCP, 2, RPAD], F8E4)
            lalo_sb = persist.tile([128, NCP, 2, RPAD], F8E4)
            lb_qk_sb = persist.tile([RANK, 2 * C], BF16)
            lb_v_sb = persist.tile([RANK, C], BF16)
            la_proj_sb = persist.tile([128, NCH, RANK], BF16)
            lb_proj_sb = persist.tile([RANK, C], BF16)
            ones1_sb = persist.tile([1, 128], F32R)

            nc.vector.memset(vres[:, :, :, D : D + 1], 1.0)  # ones column

            # --- PSUM pools: 4 (yu / r_bc) + 2x2 (score duos) = 8 banks
            ps_y = ctx.enter_context(tc.tile_pool(name="ps_y", bufs=4, space="PSUM"))
            ps_s = ctx.enter_context(tc.tile_pool(name="ps_s", bufs=2, space="PSUM"))

            small = ctx.enter_context(tc.tile_pool(name="small", bufs=1))
            pt_pool = ctx.enter_context(tc.tile_pool(name="pt", bufs=10))
            ysb_pool = ctx.enter_context(tc.tile_pool(name="ysb", bufs=3))

            # ===== attention helpers (phase-A-pool-free) =======================
            pending = []

            def _emit_division(dqb, dp, dyus):
                dqb_sl = slice(dqb * 512, (dqb + 1) * 512)
                for hi in range(2):
                    yu = dyus[hi]
                    ysb = ysb_pool.tile([D + 1, 512], F32R, tag="ysb")
                    nc.vector.tensor_copy(ysb[:], yu[:])
                    r_tmp = small.tile([1, 512], F32R, tag="rtmp")
                    nc.vector.reciprocal(r_tmp[:], ysb[D : D + 1, :])
                    r_bc = ps_y.tile([128, 512], F32, tag="y")
                    nc.tensor.matmul(
                        r_bc[:], ones1_sb[:], r_tmp[:], start=True, stop=True
                    )
                    rows = slice(64 * hi, 64 * hi + 64)
                    nc.vector.tensor_mul(
                        y_acc[rows, dp, dqb_sl], ysb[0:D, :], r_bc[rows, :]
                    )

            def emit_attention_qb(qb):
                ktl = _ktiles_for_qblock(qb)
                for p in range(NCH):  # c-chunk = head pair (2p, 2p+1)
                    yus = [
                        ps_y.tile([D + 1, 512], F32, tag="y", name=f"yu_{qb}_{p}_{i}")
                        for i in range(2)
                    ]
                    for ki, (region, j) in enumerate(ktl):
                        qts = _active_qts(region, j, qb)
                        qlo, qw = qts[0], len(qts)
                        q_sl = slice(128 * qlo, 128 * (qlo + qw))
                        rel_sl = slice(
                            128 * (qlo - 4 * qb), 128 * (qlo - 4 * qb + qw)
                        )
                        nq = 128 * qw
                        kbase = (0 if region == "e1" else T) + 128 * j
                        st_glob = kbase // 128

                        if ki == 1 and pending:
                            _emit_division(*pending.pop(0))

                        sp = ps_s.tile([128, 2, 512], F32, tag="s")
                        for hi in range(2):
                            lo = 64 * hi
                            nc.tensor.matmul(
                                sp[:, hi, 0:nq],
                                ktsb[lo : lo + 64, p, kbase : kbase + 128],
                                qT_sb[lo : lo + 64, p, q_sl],
                                start=True,
                                stop=True,
                            )
                        pt = pt_pool.tile([128, 2, 512], BF16, tag="pt")
                        nc.scalar.activation(
                            pt[:, :, 0:nq],
                            sp[:, :, 0:nq],
                            mybir.ActivationFunctionType.Exp,
                        )
                        for qt in qts:
                            if (region, j, qt) in MASK_IDX:
                                mi = MASK_IDX[(region, j, qt)]
                                rel = slice(128 * (qt - qlo), 128 * (qt - qlo + 1))
                                mb = mask_sb[:, mi : mi + 1, :].broadcast_to(
                                    [128, 2, 128]
                                )
                                nc.vector.tensor_mul(pt[:, :, rel], pt[:, :, rel], mb)
                        for hi in range(2):
                            nc.tensor.matmul(
                                yus[hi][:, rel_sl],
                                vres[:, st_glob, 2 * p + hi, :],
                                pt[:, hi, 0:nq],
                                start=(ki == 0),
                                stop=(ki == len(ktl) - 1),
                                skip_group_check=True,
                            )
                    pending.append((qb, p, _division_stage1(yus)))

            # ===== phase A: projections (compensated fp8 DoubleRow) ============
            def dr_terms(w8_sb, wlo_sb, x8_t, xlo_t):
                return ((w8_sb, x8_t), (wlo_sb, x8_t), (w8_sb, xlo_t))

            def mid_group(out_ap, x8_t, xlo_t):
                """attn-lora mid: sum_c A[c, :]^T x[c, :] -> [RPAD, 512]."""
                i = 0
                for cp in range(NCP):
                    for lh, rh in dr_terms(la8_sb, lalo_sb, x8_t, xlo_t):
                        nc.tensor.matmul(
                            out_ap,
                            lh[:, cp, :, :],
                            rh[:, cp, :, :],
                            start=(i == 0),
                            stop=(i == 3 * NCP - 1),
                            perf_mode=DR,
                        )
                        i += 1

            with tc.tile_pool(name="wk", bufs=1) as wk_pool, tc.tile_pool(
                name="wv", bufs=1
            ) as wv_pool, tc.tile_pool(name="xa", bufs=2) as xa_pool:
                wk8_sb = wk_pool.tile([128, NCP, 2, C], F8E4)
                wklo_sb = wk_pool.tile([128, NCP, 2, C], F8E4)
                wv8_sb = wv_pool.tile([128, NCP, 2, C], F8E4)
                wvlo_sb = wv_pool.tile([128, NCP, 2, C], F8E4)
                nc.sync.dma_start(out=wk8_sb[:], in_=wk8[:])
                nc.sync.dma_start(out=wklo_sb[:], in_=wklo[:])
                nc.sync.dma_start(out=wv8_sb[:], in_=wv8[:])
                nc.sync.dma_start(out=wvlo_sb[:], in_=wvlo[:])

                # ---- Q^T projection (own T rows), resident ----
                with tc.tile_pool(name="wq", bufs=1) as wq_pool:
                    wq8_sb = wq_pool.tile([128, NCP, 2, C], F8E4)
                    wqlo_sb = wq_pool.tile([128, NCP, 2, C], F8E4)
                    nc.sync.dma_start(out=wq8_sb[:], in_=wq8[:])
                    nc.sync.dma_start(out=wqlo_sb[:], in_=wqlo[:])
                    for s in range(2):
                        sl = slice(s * 512, (s + 1) * 512)
                        xq8_t = xa_pool.tile([128, NCP, 2, 512], F8E4, tag="x8")
                        xqlo_t = xa_pool.tile([128, NCP, 2, 512], F8E4, tag="xlo")
                        nc.sync.dma_start(out=xq8_t[:], in_=xq8[:, :, :, sl])
                        nc.sync.dma_start(out=xqlo_t[:], in_=xqlo[:, :, :, sl])
                        tmq_ps = ps_s.tile([128, 2, 512], F32, tag="s")
                        mid_group(tmq_ps[0:RPAD, 0, :], xq8_t, xqlo_t)
                        tmq_sb = small.tile([RANK, 512], BF16, tag="tmq")
                        nc.vector.tensor_scalar_mul(
                            tmq_sb[:], tmq_ps[0:RANK, 0, :], 1.0 / WSC
                        )
                        for mp in range(4):
                            qps = ps_s.tile([128, 2, 512], F32, tag="s")
                            for h2 in range(2):
                                m = 2 * mp + h2
                                cols = slice(128 * m, 128 * (m + 1))
                                i = 0
                                for lh, rh in dr_terms(
                                    wq8_sb, wqlo_sb, xq8_t, xqlo_t
                                ):
                                    for cp in range(NCP):
                                        nc.tensor.matmul(
                                            qps[:, h2, :],
                                            lh[:, cp, :, cols],
                                            rh[:, cp, :, :],
                                            start=(i == 0),
                                            stop=False,
                                            perf_mode=DR,
                                        )
                                        i += 1
                                nc.tensor.matmul(
                                    qps[:, h2, :],
                                    lb_qk_sb[:, cols],
                                    tmq_sb[:],
                                    start=False,
                                    stop=True,
                                )
                            nc.scalar.mul(
                                qT_sb[:, 2 * mp : 2 * mp + 2, sl], qps[:], 1.0 / WSC
                            )

                # ---- K^T and V per seq block ----
                def emit_kv_block(s, stage_on_act):
                    sl = slice(s * 512, (s + 1) * 512)
                    x8_t = xa_pool.tile([128, NCP, 2, 512], F8E4, tag="x8")
                    xlo_t = xa_pool.tile([128, NCP, 2, 512], F8E4, tag="xlo")
                    nc.sync.dma_start(out=x8_t[:], in_=x8[:, :, :, sl])
                    nc.sync.dma_start(out=xlo_t[:], in_=xlo[:, :, :, sl])
                    tsl = None
                    if s >= 2:  # e2 rows: attn lora mid
                        tsl = slice((s - 2) * 512, (s - 1) * 512)
                        tmp_ps = ps_s.tile([128, 2, 512], F32, tag="s")
                        mid_group(tmp_ps[0:RPAD, 0, :], x8_t, xlo_t)
                        nc.vector.tensor_scalar_mul(
                            tmp_kv[:, tsl], tmp_ps[0:RANK, 0, :], 1.0 / WSC
                        )
                    for mp in range(4):  # kcol tile pairs
                        kps = ps_s.tile([128, 2, 512], F32, tag="s")
                        for h2 in range(2):
                            m = 2 * mp + h2
                            cols = slice(128 * m, 128 * (m + 1))
                            i = 0
                            for lh, rh in dr_terms(wk8_sb, wklo_sb, x8_t, xlo_t):
                                for cp in range(NCP):
                                    nc.tensor.matmul(
                                        kps[:, h2, :],
                                        lh[:, cp, :, cols],
                                        rh[:, cp, :, :],
                                        start=(i == 0),
                                        stop=(i == 3 * NCP - 1 and s < 2),
                                        perf_mode=DR,
                                    )
                                    i += 1
                            if s >= 2:
                                nc.tensor.matmul(
                                    kps[:, h2, :],
                                    lb_qk_sb[:, C + 128 * m : C + 128 * (m + 1)],
                                    tmp_kv[:, tsl],
                                    start=False,
                                    stop=True,
                                )
                        dst = ktsb[:, 2 * mp : 2 * mp + 2, sl]
                        if stage_on_act:
                            nc.scalar.mul(dst, kps[:], 1.0 / WSC)
                        else:
                            nc.vector.tensor_scalar_mul(dst, kps[:], 1.0 / WSC)
                    for st in range(4):  # V: 128-row seq tiles within block
                        ssl = slice(128 * st, 128 * (st + 1))
                        vps = ps_s.tile([128, 2, 512], F32, tag="s")
                        for vc in range(2):
                            vsl = slice(512 * vc, 512 * (vc + 1))
                            i = 0
                            for lh, rh in (
                                (x8_t, wv8_sb), (xlo_t, wv8_sb), (x8_t, wvlo_sb),
                            ):
                                for cp in range(NCP):
                                    nc.tensor.matmul(
                                        vps[:, vc, :],
                                        lh[:, cp, :, ssl],
                                        rh[:, cp, :, vsl],
                                        start=(i == 0),
                                        stop=(i == 3 * NCP - 1 and s < 2),
                                        perf_mode=DR,
                                    )
                                    i += 1
                            if s >= 2:
                                base = (s - 2) * 512 + 128 * st
                                nc.tensor.matmul(
                                    vps[:, vc, :],
                                    tmp_kv[:, base : base + 128],
                                    lb_v_sb[:, vsl],
                                    start=False,
                                    stop=True,
                                )
                        dst = vres[:, 4 * s + st, :, 0:D]
                        vsrc = vps[:].rearrange("p v (h d) -> p (v h) d", h=8)
                        if stage_on_act:
                            nc.scalar.mul(dst, vsrc, 1.0 / WSC)
                        else:
                            nc.vector.tensor_scalar_mul(dst, vsrc, 1.0 / WSC)

                emit_kv_block(0, True)
                emit_kv_block(1, True)
                emit_kv_block(2, True)
                emit_attention_qb(0)
                emit_kv_block(3, False)

            # ===== phase B: output projection + qb1 ===========================
            bpool = ctx.enter_context(tc.tile_pool(name="bpool", bufs=1))
            wproj_sb = bpool.tile([128, NCH, C], BF16)
            nc.sync.dma_start(out=wproj_sb[:], in_=wproj[:])
            ost_pool = ctx.enter_context(tc.tile_pool(name="ost", bufs=2))

            def emit_proj_qb(qb):
                while pending:
                    _emit_division(*pending.pop(0))
                qb_sl = slice(qb * 512, (qb + 1) * 512)
                tm2_ps = ps_s.tile([128, 2, 512], F32, tag="s")
                for ch in range(NCH):
                    nc.tensor.matmul(
                        tm2_ps[0:RANK, 0, :],
                        la_proj_sb[:, ch, :],
                        y_acc[:, ch, qb_sl],
                        start=(ch == 0),
                        stop=(ch == NCH - 1),
                    )
                tm2_sb = small.tile([RANK, 512], BF16, tag="tm2")
                nc.vector.tensor_copy(tm2_sb[:], tm2_ps[0:RANK, 0, :])
                for qs in range(4):
                    qrow = 512 * qb + 128 * qs
                    ops = ps_s.tile([128, 2, 512], F32, tag="s")
                    for co in range(2):
                        cos = slice(512 * co, 512 * (co + 1))
                        for ch in range(NCH):
                            nc.tensor.matmul(
                                ops[:, co, :],
                                y_acc[:, ch, qrow : qrow + 128],
                                wproj_sb[:, ch, cos],
                                start=(ch == 0),
                                stop=False,
                            )
                        nc.tensor.matmul(
                            ops[:, co, :],
                            tm2_sb[:, 128 * qs : 128 * (qs + 1)],
                            lb_proj_sb[:, cos],
                            start=False,
                            stop=True,
                        )
                    ost = ost_pool.tile([128, 2, 512], F32, tag="ost")
                    nc.vector.tensor_copy(ost[:], ops[:])
                    nc.sync.dma_start(
                        out=yout[qrow : qrow + 128, :],
                        in_=ost[:].rearrange("p a b -> p (a b)"),
                    )

            emit_proj_qb(0)
            emit_attention_qb(1)
            emit_proj_qb(1)
    return nc


_PROGRAM = None


def _get_program():
    global _PROGRAM
    if _PROGRAM is None:
        _PROGRAM = _build_program()
    return _PROGRAM


# ---------------------------------------------------------------------------
# Host side
# ---------------------------------------------------------------------------
def _delayed_mask_np(t):
    ones = np.ones((t, t), dtype=bool)
    m11 = np.tril(ones) & np.triu(ones, -(LOOKAHEAD + OVERLAP))
    m12 = np.tril(ones, -LOOKAHEAD)
    m21 = np.tril(ones, LOOKAHEAD) & np.triu(ones, -OVERLAP)
    m22 = np.tril(ones)
    return np.block([[m11, m12], [m21, m22]])


def _fp8_pair(a):
    hi = a.astype(FP8NP)
    lo = (a - hi.astype(np.float32)).astype(FP8NP)
    return hi, lo


def _cp_layout(m):
    """[C, N] -> [128, NCP, 2, N] with c = 256*cp + 128*i + p."""
    n = m.shape[1]
    return np.ascontiguousarray(m.reshape(NCP, 2, 128, n).transpose(2, 0, 1, 3))


def kernel(
    e1,
    e2,
    W_attn,
    W_proj,
    lora_A_attn,
    lora_B_attn,
    lora_A_proj,
    lora_B_proj,
    _trace=False,
):
    f32 = np.float32
    bf16 = ml_dtypes.bfloat16
    e1 = np.asarray(e1, f32)
    e2 = np.asarray(e2, f32)
    W_attn = np.asarray(W_attn, f32)
    W_proj = np.asarray(W_proj, f32)
    lora_A_attn = np.asarray(lora_A_attn, f32)
    lora_B_attn = np.asarray(lora_B_attn, f32)
    lora_A_proj = np.asarray(lora_A_proj, f32)
    lora_B_proj = np.asarray(lora_B_proj, f32)
    nc = _get_program()
    M = _delayed_mask_np(T)

    # --- role-independent prep (once) ---
    wq8, wqlo = _fp8_pair(_cp_layout(W_attn[:, :C] * (WSC * QSCALE)))
    wk8, wklo = _fp8_pair(_cp_layout(W_attn[:, C : 2 * C] * WSC))
    wv8, wvlo = _fp8_pair(_cp_layout(W_attn[:, 2 * C :] * WSC))
    la_pad = np.zeros((C, RPAD), f32)
    la_pad[:, :RANK] = lora_A_attn * WSC
    la8, lalo = _fp8_pair(_cp_layout(la_pad))
    la_proj = np.ascontiguousarray(
        lora_A_proj.reshape(NCH, 128, RANK).transpose(1, 0, 2)
    ).astype(bf16)
    wproj_r = np.ascontiguousarray(
        W_proj.reshape(NCH, 128, C).transpose(1, 0, 2)
    ).astype(bf16)
    lb_v = (np.ascontiguousarray(lora_B_attn[:, 2 * C :]) * (LSCALE * WSC)).astype(
        bf16
    )
    ones1 = np.ones((1, 128), f32)

    lbqk = {}
    lbp = {}
    for r in (0, 1):
        q = np.array(lora_B_attn[:, :C], dtype=f32) * (LSCALE * WSC * QSCALE)
        if r == 0:
            q[:] = 0.0
        k = lora_B_attn[:, C : 2 * C] * (LSCALE * WSC)
        lbqk[r] = np.concatenate([q, k], axis=1).astype(bf16)
        p = np.array(lora_B_proj, dtype=f32) * LSCALE
        if r == 0:
            p[:] = 0.0
        lbp[r] = p.astype(bf16)

    masks_r = {}
    for r in (0, 1):
        mk = np.empty((NMASK, 128, 128), dtype=bf16)
        for i, (region, j, qt) in enumerate(MASK_TILES):
            qg = r * T + 128 * qt
            kg = (0 if region == "e1" else T) + 128 * j
            mk[i] = M[qg : qg + 128, kg : kg + 128].T.astype(f32)
        masks_r[r] = mk

    in_maps = []
    x_cache = None
    for core in range(8):
        b, r = core // 2, core % 2
        if r == 0:
            x = np.concatenate([e1[b], e2[b]], axis=0)  # [2T, C]
            xT = np.ascontiguousarray(x.T)
            x_cache = _fp8_pair(_cp_layout(xT))
        x8b, xlob = x_cache
        qsl = slice(r * T, (r + 1) * T)
        in_maps.append({
            "x8": x8b,
            "xlo": xlob,
            "xq8": np.ascontiguousarray(x8b[:, :, :, qsl]),
            "xqlo": np.ascontiguousarray(xlob[:, :, :, qsl]),
            "wq8": wq8, "wqlo": wqlo,
            "wk8": wk8, "wklo": wklo,
            "wv8": wv8, "wvlo": wvlo,
            "la8": la8, "lalo": lalo,
            "lb_qk": lbqk[r],
            "lb_v": lb_v,
            "la_proj": la_proj,
            "lb_proj": lbp[r],
            "wproj": wproj_r,
            "masks": masks_r[r],
            "ones1": ones1,
        })

    res = run_bass_kernel_spmd(nc, in_maps, core_ids=list(range(8)), trace=_trace)
    y1 = np.stack([res.results[2 * b]["yout"] for b in range(B)])
    y2 = np.stack([res.results[2 * b + 1]["yout"] for b in range(B)])
    if _trace:
        kernel.last_results = res
    return y1, y2
